# revision 36
# baseline (speedup 1.0000x reference)
"""Trainium2 Bass kernel for nn_DAGExecutor (digit-softmax + 8-step DAG recurrence).

Fully data-parallel: 32768 (B*T) tokens sharded as 4096 tokens per core across
8 cores; no cross-core communication.  Per core tokens live as [128 partitions
x 32 tokens].

Engine split: ACT does exp/ln/abs; Pool (gpsimd) takes ~90% of the two big
phase-1 elementwise passes plus assorted multiplies; DVE owns the grouped
reduces and the small-op chains.  Phase 1 is software-pipelined (front/back
skew) because engine queues execute in program order.  Phase 2 pipelines each
step's heavy node-dots ("front", which only need state that is two steps old)
one step ahead, leaving a short per-token fixup chain on the critical path.
"""

import sys

for _p in ("/opt/trn_rl_repo",):
    if _p not in sys.path:
        sys.path.insert(0, _p)

import numpy as np

import concourse.bass as bass
import concourse.bacc as bacc
import concourse.mybir as mybir
import concourse.tile as tile
from concourse.bass_utils import run_bass_kernel_spmd

F32 = mybir.dt.float32
I32 = mybir.dt.int32
AX = mybir.AxisListType
ALU = mybir.AluOpType
ACTF = mybir.ActivationFunctionType

# ---- problem constants (hardcoded; must match reference setup) ----
B, T = 8, 4096
NI = 9          # initial nodes
DD = 8          # dag depth
TN = 17         # total nodes
DS = 8          # digit slots
BASE = 10
PTOK = 720      # values per token in digit_logits = NI*DS*BASE
NCORES = 8
TOK_PER_CORE = (B * T) // NCORES        # 4096
P = 128                                  # partitions
TP = TOK_PER_CORE // P                   # 32 tokens per partition
MAG_MIN, MAG_MAX = 1e-12, 1e28
LOG_LIM = 100.0
INV_TEMP = 100.0                         # 1/TEMP
INV_SIGN_TEMP = 1e4                      # 1/SIGN_TEMP
# f32 exp overflow boundary: clamping exp's input here yields a value big
# enough that the downstream [1e-12,1e28] clip matches the reference's inf path
EXP_CLAMP = 88.7228355
POWERS = np.asarray([float(BASE) ** (4 - 1 - d) for d in range(DS)], dtype=np.float32)
LN2 = float(np.log(np.float32(2.0)))

CH = 4                                   # tokens-per-partition per phase-1 chunk
NCHUNK = TP // CH                        # 8 chunks
GRP = CH * NI * DS                       # softmax groups per chunk (288)
CW = CH * PTOK                           # chunk width (2880)
SG = GRP // 2                            # Pool share of the subtract pass
HR = (CH * NI) // 2                      # half of the token*node axis


def _emit_ln(nc, pool, dst_ap, src_ap, width, tag):
    """dst = ln(src) for src in [1e-12, +inf] via exponent/mantissa split.

    ACT Ln only covers |x| <= 2^64 and direct rescaling costs accuracy near
    ln(x)=0; the split keeps ~2ulp everywhere."""
    xb = src_ap.bitcast(I32)
    eint = pool.tile([P, width], I32, tag=tag + "_e")
    nc.vector.tensor_scalar(eint[:], xb, 23, None, ALU.logical_shift_right)
    ef = pool.tile([P, width], F32, tag=tag + "_ef")
    nc.vector.tensor_copy(ef[:], eint[:])
    le = pool.tile([P, width], F32, tag=tag + "_le")
    nc.vector.tensor_scalar(le[:], ef[:], 127.0, LN2, ALU.subtract, ALU.mult)
    mbits = pool.tile([P, width], I32, tag=tag + "_mb")
    nc.vector.tensor_scalar(
        mbits[:], xb, 0x007FFFFF, 0x3F800000, ALU.bitwise_and, ALU.bitwise_or
    )
    lnm = pool.tile([P, width], F32, tag=tag + "_lm")
    nc.scalar.activation(lnm[:], mbits[:].bitcast(F32), ACTF.Ln)
    nc.vector.tensor_tensor(dst_ap, lnm[:], le[:], ALU.add)


def _patch_act_tables():
    """Force all activations onto the natural_log_exp_and_others table set.

    The table-load pass greedily alternates exp_and_others / natural_log,
    inserting ~18 ACT table loads (~2.7us each on HW).  Emptying every other
    set (indices preserved) makes the combined set the only candidate."""
    import concourse.hw_specs as hw_specs
    orig = hw_specs.get_activation_tables

    def patched(arch):
        tabs = orig(arch)
        keep = "natural_log_exp_and_others"
        if keep not in tabs:
            return tabs
        return {k: (v if k == keep else set()) for k, v in tabs.items()}

    patched.__wrapped__ = orig
    bacc.get_activation_tables = patched


def build_program():
    _patch_act_tables()
    nc = bacc.Bacc("TRN2", target_bir_lowering=False, debug=False)

    dl = nc.dram_tensor("dl", [P, TP * PTOK], F32, kind="ExternalInput").ap()
    vsg = nc.dram_tensor("vsg", [P, TP * TN], F32, kind="ExternalInput").ap()
    od = nc.dram_tensor("od", [P, TP * DD * TN], F32, kind="ExternalInput").ap()
    gd = nc.dram_tensor("gd", [P, TP * DD], F32, kind="ExternalInput").ap()
    wpat = nc.dram_tensor("wpat", [P, DS * BASE], F32, kind="ExternalInput").ap()
    out = nc.dram_tensor("out", [P, TP], F32, kind="ExternalOutput").ap()

    with tile.TileContext(nc) as tc:
        with (
            tc.tile_pool(name="persist", bufs=1) as pp,
            tc.tile_pool(name="xin", bufs=4) as xp,
            tc.tile_pool(name="dbuf", bufs=3) as dp,
            tc.tile_pool(name="small", bufs=3) as sp,
            tc.tile_pool(name="steps", bufs=1) as stp,
            tc.tile_pool(name="steps2", bufs=2) as st2,
        ):
            # ---- persistent tiles ----
            vmag = pp.tile([P, TP * NI], F32, tag="vmag")
            otile = pp.tile([P, TP * DD * TN], F32, tag="otile")
            wsign = pp.tile([P, TP * TN], F32, tag="wsign")
            gtile = pp.tile([P, TP * DD], F32, tag="gtile")
            wtile = pp.tile([P, DS * BASE], F32, tag="wtile")
            absO2 = pp.tile([P, TP * DD * TN], F32, tag="absO2")
            onemg = pp.tile([P, TP * DD], F32, tag="onemg")
            wmag = pp.tile([P, TP * TN], F32, tag="wmag")
            signed = pp.tile([P, TP * TN], F32, tag="signed")
            logm = pp.tile([P, TP * TN], F32, tag="logm")
            denall = pp.tile([P, NCHUNK * GRP], F32, tag="denall")
            numall = pp.tile([P, NCHUNK * GRP], F32, tag="numall")

            nc.sync.dma_start(wtile[:], wpat)

            # ---- phase 1: digit softmax expected value -> vmag ----
            front_state = {}

            def p1_fd(ci):
                """DVE-side front: dma, group max, DVE share of subtract."""
                x = xp.tile([P, CW], F32, tag="x")
                nc.sync.dma_start(x[:], dl[:, ci * CW:(ci + 1) * CW])
                xv = x[:].rearrange("p (g b) -> p g b", b=BASE)
                m = sp.tile([P, GRP], F32, tag="m")
                nc.vector.tensor_reduce(m[:], xv, AX.X, ALU.max)
                d = dp.tile([P, CW], F32, tag="d")
                dv = d[:].rearrange("p (g b) -> p g b", b=BASE)
                mb = m[:].unsqueeze(2).broadcast_to((P, GRP, BASE))
                nc.vector.tensor_tensor(dv[:, SG:], xv[:, SG:], mb[:, SG:], ALU.subtract)
                front_state[ci] = (x, m, d)

            def p1_fp(ci):
                """Pool-side front: Pool share of subtract."""
                x, m, d = front_state[ci]
                xv = x[:].rearrange("p (g b) -> p g b", b=BASE)
                dv = d[:].rearrange("p (g b) -> p g b", b=BASE)
                mb = m[:].unsqueeze(2).broadcast_to((P, GRP, BASE))
                nc.gpsimd.tensor_tensor(dv[:, :SG], xv[:, :SG], mb[:, :SG], ALU.subtract)

            def p1_e(ci):
                """exp halves; the DVE-subtract half (upper) is ready first."""
                x, m, d = front_state[ci]
                e = xp.tile([P, CW], F32, tag="x")
                HW = CW // 2
                nc.scalar.activation(e[:, HW:], d[:, HW:], ACTF.Exp, scale=INV_TEMP)
                nc.scalar.activation(e[:, :HW], d[:, :HW], ACTF.Exp, scale=INV_TEMP)
                front_state[ci] = e

            def p1_bp(ci):
                """Pool back: weight-mult halves (early-e half first)."""
                e = front_state[ci]
                w = dp.tile([P, CW], F32, tag="d")
                wv = w[:].rearrange("p (r q) -> p r q", q=DS * BASE)
                ev8 = e[:].rearrange("p (r q) -> p r q", q=DS * BASE)
                wb = wtile[:].unsqueeze(1).broadcast_to((P, CH * NI, DS * BASE))
                nc.gpsimd.tensor_tensor(wv[:, HR:], ev8[:, HR:], wb[:, HR:], ALU.mult)
                nc.gpsimd.tensor_tensor(wv[:, :HR], ev8[:, :HR], wb[:, :HR], ALU.mult)
                front_state[ci] = (e, w)

            def p1_bd(ci):
                """DVE back: den/num reduces (early halves first) + reciprocal."""
                e, w = front_state.pop(ci)
                den = denall[:, ci * GRP:(ci + 1) * GRP]
                ev = e[:].rearrange("p (g b) -> p g b", b=BASE)
                HG = GRP // 2
                nc.vector.tensor_reduce(den[:, HG:], ev[:, HG:], AX.X, ALU.add)
                nc.vector.tensor_reduce(den[:, :HG], ev[:, :HG], AX.X, ALU.add)
                num = numall[:, ci * GRP:(ci + 1) * GRP]
                w3 = w[:].rearrange("p (g b) -> p g b", b=BASE)
                nc.vector.tensor_reduce(num[:, HG:], w3[:, HG:], AX.X, ALU.add)
                nc.vector.tensor_reduce(num[:, :HG], w3[:, :HG], AX.X, ALU.add)
                rcp = sp.tile([P, GRP], F32, tag="rcp")
                scr = sp.tile([P, GRP], F32, tag="scr")
                nc.vector.reciprocal_approx_accurate(rcp[:], den[:], scr[:])
                return rcp

            def p1_tail(ci, rcp):
                """expected value + pow-weighted sum (Pool) + clip (DVE)."""
                num = numall[:, ci * GRP:(ci + 1) * GRP]
                ex = sp.tile([P, GRP], F32, tag="ex")
                nc.gpsimd.tensor_tensor(ex[:], num, rcp[:], ALU.mult)
                ex3 = ex[:].rearrange("p (r d) -> p r d", d=DS)
                v4 = sp.tile([P, CH * NI * 4], F32, tag="v4")
                v43 = v4[:].rearrange("p (r d) -> p r d", d=4)
                nc.gpsimd.tensor_tensor(v43, ex3[:, :, 0:4], ex3[:, :, 4:8], ALU.add)
                v2 = sp.tile([P, CH * NI * 2], F32, tag="v2")
                v23 = v2[:].rearrange("p (r d) -> p r d", d=2)
                nc.gpsimd.tensor_tensor(v23, v43[:, :, 0:2], v43[:, :, 2:4], ALU.add)
                vm = sp.tile([P, CH * NI], F32, tag="vm")
                nc.gpsimd.tensor_tensor(vm[:], v23[:, :, 0], v23[:, :, 1], ALU.add)
                nc.vector.tensor_scalar(
                    vmag[:, ci * CH * NI:(ci + 1) * CH * NI], vm[:],
                    MAG_MIN, MAG_MAX, ALU.max, ALU.min,
                )

            p1_fd(0)
            p1_fp(0)
            p1_e(0)
            p1_fd(1)
            # phase-2 inputs after the first chunk DMAs so they don't block them
            nc.sync.dma_start(otile[:], od)
            nc.sync.dma_start(wsign[:], vsg)
            nc.sync.dma_start(gtile[:], gd)
            nc.scalar.activation(absO2[:], otile[:], ACTF.Abs, scale=2.0)
            nc.vector.tensor_scalar(onemg[:], gtile[:], -1.0, 1.0, ALU.mult, ALU.add)
            for ci in range(NCHUNK):
                p1_bp(ci)
                rcp = p1_bd(ci)
                if ci + 1 < NCHUNK:
                    p1_fp(ci + 1)
                    p1_e(ci + 1)
                if ci + 2 < NCHUNK:
                    p1_fd(ci + 2)
                p1_tail(ci, rcp)

            # ---- phase 2: DAG recurrence ----
            nc.gpsimd.memset(wmag[:], MAG_MIN)
            wm3 = wmag[:].rearrange("p (t n) -> p t n", n=TN)
            ws3 = wsign[:].rearrange("p (t n) -> p t n", n=TN)
            sg3 = signed[:].rearrange("p (t n) -> p t n", n=TN)
            lg3 = logm[:].rearrange("p (t n) -> p t n", n=TN)
            o4 = otile[:].rearrange("p (t s n) -> p t s n", s=DD, n=TN)
            a4 = absO2[:].rearrange("p (t s n) -> p t s n", s=DD, n=TN)

            nc.vector.tensor_copy(
                wm3[:, :, 0:NI], vmag[:].rearrange("p (t n) -> p t n", n=NI)
            )
            nc.vector.tensor_tensor(signed[:], wsign[:], wmag[:], ALU.mult)
            _emit_ln(nc, stp, logm[:], wmag[:], TP * TN, "lni")

            def gs(ap, s, n_bcast=None):
                v = ap[:, s::DD]
                if n_bcast is None:
                    return v
                return v.unsqueeze(2).broadcast_to((P, TP, n_bcast))

            def p2_front(s):
                """Heavy dots over nodes [0, K): everything but the node written
                by step s-1.  Depends only on >=2-step-old state, so it runs one
                step ahead of its consumer."""
                K = NI if s == 0 else NI - 1 + s
                t1 = stp.tile([P, TP * TN], F32, tag="t1f")
                t13 = t1[:].rearrange("p (t n) -> p t n", n=TN)
                nc.gpsimd.tensor_tensor(
                    t13[:, :, :K], lg3[:, :, :K], gs(onemg[:], s, K), ALU.mult
                )
                t2 = stp.tile([P, TP * TN], F32, tag="t2f")
                t23 = t2[:].rearrange("p (t n) -> p t n", n=TN)
                nc.vector.tensor_tensor(
                    t23[:, :, :K], sg3[:, :, :K], gs(gtile[:], s, K), ALU.mult
                )
                mx = stp.tile([P, TP * TN], F32, tag="mxf")
                mx3 = mx[:].rearrange("p (t n) -> p t n", n=TN)
                nc.vector.tensor_tensor(
                    mx3[:, :, :K], t13[:, :, :K], t23[:, :, :K], ALU.add
                )
                rt = stp.tile([P, TP * TN], F32, tag="rtf")
                rt3 = rt[:].rearrange("p (t n) -> p t n", n=TN)
                nc.vector.tensor_tensor(
                    rt3[:, :, :K], mx3[:, :, :K], o4[:, :, s, :K], ALU.mult
                )
                rold = st2.tile([P, TP], F32, tag="rold")
                nc.vector.tensor_reduce(rold[:], rt3[:, :, :K], AX.X, ALU.add)

                sw = stp.tile([P, TP * TN], F32, tag="swf")
                sw3 = sw[:].rearrange("p (t n) -> p t n", n=TN)
                nc.gpsimd.tensor_tensor(
                    sw3[:, :, :K], ws3[:, :, :K], a4[:, :, s, :K], ALU.mult
                )
                swp = stp.tile([P, TP * TN], F32, tag="swpf")
                swp3 = swp[:].rearrange("p (t n) -> p t n", n=TN)
                nc.scalar.activation(swp3[:, :, :K], sw3[:, :, :K], ACTF.Copy, bias=1.0)
                # product over the K nodes: pairwise multiply tree on Pool
                ta = st2.tile([P, TP * TN], F32, tag="ta")
                tb = st2.tile([P, TP * TN], F32, tag="tb")
                pbufs = [
                    ta[:].rearrange("p (t n) -> p t n", n=TN),
                    tb[:].rearrange("p (t n) -> p t n", n=TN),
                ]
                src3, width, pi = swp3, K, 0
                while width > 1:
                    half, odd = width // 2, width % 2
                    dst3 = pbufs[pi]
                    nc.gpsimd.tensor_tensor(
                        dst3[:, :, :half], src3[:, :, :half],
                        src3[:, :, half:2 * half], ALU.mult,
                    )
                    if odd:
                        nc.vector.tensor_copy(dst3[:, :, half], src3[:, :, 2 * half])
                    src3, width, pi = dst3, half + odd, 1 - pi
                return rold, src3[:, :, 0]

            res = sp.tile([P, TP], F32, tag="res")
            fr = p2_front(0)

            for s in range(DD):
                last = s == DD - 1
                rold, prodold = fr

                rp = st2.tile([P, 2 * TP], F32, tag="rp")  # [R | prod]
                if s == 0:
                    nc.vector.tensor_copy(rp[:, 0:TP], rold[:])
                    nc.vector.tensor_copy(rp[:, TP:2 * TP], prodold)
                else:
                    # fold in the node written by step s-1 (index NI-1+s)
                    nd = NI - 1 + s
                    q1 = stp.tile([P, TP], F32, tag="q1")
                    nc.vector.tensor_tensor(
                        q1[:], lg3[:, :, nd], gs(onemg[:], s), ALU.mult
                    )
                    q2 = stp.tile([P, TP], F32, tag="q2")
                    nc.vector.tensor_tensor(
                        q2[:], sg3[:, :, nd], gs(gtile[:], s), ALU.mult
                    )
                    mixn = stp.tile([P, TP], F32, tag="mixn")
                    nc.vector.tensor_tensor(mixn[:], q1[:], q2[:], ALU.add)
                    rn = stp.tile([P, TP], F32, tag="rn")
                    nc.vector.tensor_tensor(rn[:], mixn[:], o4[:, :, s, nd], ALU.mult)
                    nc.vector.tensor_tensor(rp[:, 0:TP], rold[:], rn[:], ALU.add)
                    swn = stp.tile([P, TP], F32, tag="swn")
                    nc.vector.tensor_tensor(swn[:], ws3[:, :, nd], a4[:, :, s, nd], ALU.mult)
                    nc.vector.tensor_scalar(swn[:], swn[:], 1.0, None, ALU.add)
                    nc.vector.tensor_tensor(rp[:, TP:2 * TP], prodold, swn[:], ALU.mult)

                # prefetch next step's heavy dots while this step's tail runs
                if not last:
                    fr = p2_front(s + 1)

                # tanh(y/SIGN_TEMP) = 1 - 2/(1+exp(2e4*y)) on [R | prod] at once
                yc = stp.tile([P, 2 * TP], F32, tag="yc")
                nc.vector.tensor_scalar(yc[:], rp[:], -0.005, 0.005, ALU.max, ALU.min)
                tE = stp.tile([P, 2 * TP], F32, tag="tE")
                nc.scalar.activation(tE[:], yc[:], ACTF.Exp, scale=2.0 * INV_SIGN_TEMP)
                u = stp.tile([P, 2 * TP], F32, tag="u")
                nc.vector.tensor_scalar(u[:], tE[:], 1.0, 1e30, ALU.add, ALU.min)
                rc2 = stp.tile([P, 2 * TP], F32, tag="rc2")
                sc2 = stp.tile([P, 2 * TP], F32, tag="sc2")
                nc.vector.reciprocal_approx_accurate(rc2[:], u[:], sc2[:])
                th = stp.tile([P, 2 * TP], F32, tag="th")
                nc.vector.tensor_scalar(th[:], rc2[:], -2.0, 1.0, ALU.mult, ALU.add)
                lin_sign, log_sign = th[:, 0:TP], th[:, TP:2 * TP]

                # s_new = G*lin + (1-G)*log  (clip dropped: convex combo of
                # values in [-1,1] stays within 1ulp of the range)
                sa = stp.tile([P, TP], F32, tag="sa")
                nc.vector.tensor_tensor(sa[:], lin_sign, gs(gtile[:], s), ALU.mult)
                sb = stp.tile([P, TP], F32, tag="sb")
                nc.vector.tensor_tensor(sb[:], log_sign, gs(onemg[:], s), ALU.mult)
                snew = stp.tile([P, TP], F32, tag="snew")
                nc.vector.tensor_tensor(snew[:], sa[:], sb[:], ALU.add)
                nc.vector.tensor_scalar(snew[:], snew[:], -1.0, 1.0, ALU.max, ALU.min)

                # m_new = G*min(|R|,MAX) + (1-G)*exp(clip(R,-100,EXP_CLAMP)), clipped.
                # The EXP_CLAMP upper bound replaces the reference's inf path: any
                # clamped value yields (1-G)*e^88.72 >= 2e31, which the final clip
                # maps to 1e28 exactly as inf would.
                R = rp[:, 0:TP]
                absR = stp.tile([P, TP], F32, tag="absR")
                nc.scalar.activation(absR[:], R, ACTF.Abs)
                nc.vector.tensor_scalar(absR[:], absR[:], MAG_MAX, None, ALU.min)
                rc = stp.tile([P, TP], F32, tag="rc")
                nc.vector.tensor_scalar(rc[:], R, -LOG_LIM, EXP_CLAMP, ALU.max, ALU.min)
                logres = stp.tile([P, TP], F32, tag="logres")
                nc.scalar.activation(logres[:], rc[:], ACTF.Exp)
                ma = stp.tile([P, TP], F32, tag="ma")
                nc.vector.tensor_tensor(ma[:], absR[:], gs(gtile[:], s), ALU.mult)
                mb2 = stp.tile([P, TP], F32, tag="mb2")
                nc.vector.tensor_tensor(mb2[:], logres[:], gs(onemg[:], s), ALU.mult)
                mnew = stp.tile([P, TP], F32, tag="mnew")
                nc.vector.tensor_tensor(mnew[:], ma[:], mb2[:], ALU.add)
                nc.vector.tensor_scalar(mnew[:], mnew[:], MAG_MIN, MAG_MAX, ALU.max, ALU.min)

                if last:
                    # output = wsign[16] * wmag[16]; skip the state writes
                    nc.vector.tensor_tensor(res[:], mnew[:], snew[:], ALU.mult)
                else:
                    ni = NI + s
                    nc.vector.tensor_copy(wm3[:, :, ni], mnew[:])
                    nc.vector.tensor_copy(ws3[:, :, ni], snew[:])
                    nc.gpsimd.tensor_tensor(sg3[:, :, ni], mnew[:], snew[:], ALU.mult)
                    _emit_ln(nc, stp, lg3[:, :, ni], mnew[:], TP, "lns")

            nc.sync.dma_start(out, res[:])

    nc.compile()
    return nc


_NC_CACHE = None


def _get_nc():
    global _NC_CACHE
    if _NC_CACHE is None:
        _NC_CACHE = build_program()
    return _NC_CACHE


def make_in_maps(digit_logits, V_sign, O, G):
    dlf = np.ascontiguousarray(digit_logits, dtype=np.float32).reshape(B * T, PTOK)
    vsf = np.ascontiguousarray(V_sign, dtype=np.float32).reshape(B * T, TN)
    of = np.ascontiguousarray(O, dtype=np.float32).reshape(B * T, DD * TN)
    gf = np.ascontiguousarray(G, dtype=np.float32).reshape(B * T, DD)
    pat = np.zeros(DS * BASE, dtype=np.float32)
    for dd in range(DS):
        for i in range(BASE):
            pat[dd * BASE + i] = i * POWERS[dd]
    wpat = np.tile(pat[None, :], (P, 1))
    in_maps = []
    for c in range(NCORES):
        s0, s1 = c * TOK_PER_CORE, (c + 1) * TOK_PER_CORE
        in_maps.append({
            "dl": np.ascontiguousarray(dlf[s0:s1].reshape(P, TP * PTOK)),
            "vsg": np.ascontiguousarray(vsf[s0:s1].reshape(P, TP * TN)),
            "od": np.ascontiguousarray(of[s0:s1].reshape(P, TP * DD * TN)),
            "gd": np.ascontiguousarray(gf[s0:s1].reshape(P, TP * DD)),
            "wpat": wpat,
        })
    return in_maps


def kernel(digit_logits, V_sign, O, G, _trace=False, _return_results=False):
    nc = _get_nc()
    in_maps = make_in_maps(digit_logits, V_sign, O, G)
    res = run_bass_kernel_spmd(nc, in_maps, list(range(NCORES)), trace=_trace)
    outs = [np.asarray(res.results[c]["out"]).reshape(TOK_PER_CORE) for c in range(NCORES)]
    full = np.concatenate(outs).reshape(B, T)
    if _return_results:
        return full, res
    return full


# revision 37
# speedup vs baseline: 1.0838x; 1.0838x over previous
"""Trainium2 Bass kernel for nn_DAGExecutor (digit-softmax + 8-step DAG recurrence).

Fully data-parallel: 32768 (B*T) tokens sharded as 4096 tokens per core across
8 cores; no cross-core communication.  Per core tokens live as [128 partitions
x 32 tokens].

Engine split: ACT does exp/ln/abs; Pool (gpsimd) takes ~90% of the two big
phase-1 elementwise passes plus assorted multiplies; DVE owns the grouped
reduces and the small-op chains.  Phase 1 is software-pipelined (front/back
skew) because engine queues execute in program order.  Phase 2 pipelines each
step's heavy node-dots ("front", which only need state that is two steps old)
one step ahead, leaving a short per-token fixup chain on the critical path.
"""

import sys

for _p in ("/opt/trn_rl_repo",):
    if _p not in sys.path:
        sys.path.insert(0, _p)

import numpy as np

import concourse.bass as bass
import concourse.bacc as bacc
import concourse.mybir as mybir
import concourse.tile as tile
from concourse.bass_utils import run_bass_kernel_spmd

F32 = mybir.dt.float32
I32 = mybir.dt.int32
AX = mybir.AxisListType
ALU = mybir.AluOpType
ACTF = mybir.ActivationFunctionType

# ---- problem constants (hardcoded; must match reference setup) ----
B, T = 8, 4096
NI = 9          # initial nodes
DD = 8          # dag depth
TN = 17         # total nodes
DS = 8          # digit slots
BASE = 10
PTOK = 720      # values per token in digit_logits = NI*DS*BASE
NCORES = 8
TOK_PER_CORE = (B * T) // NCORES        # 4096
P = 128                                  # partitions
TP = TOK_PER_CORE // P                   # 32 tokens per partition
MAG_MIN, MAG_MAX = 1e-12, 1e28
LOG_LIM = 100.0
INV_TEMP = 100.0                         # 1/TEMP
INV_SIGN_TEMP = 1e4                      # 1/SIGN_TEMP
# f32 exp overflow boundary: clamping exp's input here yields a value big
# enough that the downstream [1e-12,1e28] clip matches the reference's inf path
EXP_CLAMP = 88.7228355
POWERS = np.asarray([float(BASE) ** (4 - 1 - d) for d in range(DS)], dtype=np.float32)
LN2 = float(np.log(np.float32(2.0)))

CH = 4                                   # tokens-per-partition per phase-1 chunk
NCHUNK = TP // CH                        # 8 chunks
GRP = CH * NI * DS                       # softmax groups per chunk (288)
CW = CH * PTOK                           # chunk width (2880)
SG = GRP // 2                            # Pool share of the subtract pass
HR = (CH * NI) // 2                      # half of the token*node axis


def _emit_ln(nc, pool, dst_ap, src_ap, width, tag):
    """dst = ln(src) for src in [1e-12, +inf] via exponent/mantissa split.

    ACT Ln only covers |x| <= 2^64 and direct rescaling costs accuracy near
    ln(x)=0; the split keeps ~2ulp everywhere."""
    xb = src_ap.bitcast(I32)
    eint = pool.tile([P, width], I32, tag=tag + "_e")
    nc.vector.tensor_scalar(eint[:], xb, 23, None, ALU.logical_shift_right)
    ef = pool.tile([P, width], F32, tag=tag + "_ef")
    nc.vector.tensor_copy(ef[:], eint[:])
    le = pool.tile([P, width], F32, tag=tag + "_le")
    nc.vector.tensor_scalar(le[:], ef[:], 127.0, LN2, ALU.subtract, ALU.mult)
    mbits = pool.tile([P, width], I32, tag=tag + "_mb")
    nc.vector.tensor_scalar(
        mbits[:], xb, 0x007FFFFF, 0x3F800000, ALU.bitwise_and, ALU.bitwise_or
    )
    lnm = pool.tile([P, width], F32, tag=tag + "_lm")
    nc.scalar.activation(lnm[:], mbits[:].bitcast(F32), ACTF.Ln)
    nc.vector.tensor_tensor(dst_ap, lnm[:], le[:], ALU.add)


def _patch_act_tables():
    """Force all activations onto the natural_log_exp_and_others table set.

    The table-load pass greedily alternates exp_and_others / natural_log,
    inserting ~18 ACT table loads (~2.7us each on HW).  Emptying every other
    set (indices preserved) makes the combined set the only candidate."""
    import concourse.hw_specs as hw_specs
    orig = hw_specs.get_activation_tables

    def patched(arch):
        tabs = orig(arch)
        keep = "natural_log_exp_and_others"
        if keep not in tabs:
            return tabs
        return {k: (v if k == keep else set()) for k, v in tabs.items()}

    patched.__wrapped__ = orig
    bacc.get_activation_tables = patched


def build_program():
    _patch_act_tables()
    nc = bacc.Bacc("TRN2", target_bir_lowering=False, debug=False)

    dl = nc.dram_tensor("dl", [P, TP * PTOK], F32, kind="ExternalInput").ap()
    vsg = nc.dram_tensor("vsg", [P, TP * TN], F32, kind="ExternalInput").ap()
    od = nc.dram_tensor("od", [P, TP * DD * TN], F32, kind="ExternalInput").ap()
    gd = nc.dram_tensor("gd", [P, TP * DD], F32, kind="ExternalInput").ap()
    wpat = nc.dram_tensor("wpat", [P, DS * BASE], F32, kind="ExternalInput").ap()
    out = nc.dram_tensor("out", [P, TP], F32, kind="ExternalOutput").ap()

    with tile.TileContext(nc) as tc:
        with (
            tc.tile_pool(name="persist", bufs=1) as pp,
            tc.tile_pool(name="xin", bufs=4) as xp,
            tc.tile_pool(name="dbuf", bufs=3) as dp,
            tc.tile_pool(name="small", bufs=3) as sp,
            tc.tile_pool(name="steps", bufs=1) as stp,
            tc.tile_pool(name="steps2", bufs=2) as st2,
        ):
            # ---- persistent tiles ----
            vmag = pp.tile([P, TP * NI], F32, tag="vmag")
            otile = pp.tile([P, TP * DD * TN], F32, tag="otile")
            wsign = pp.tile([P, TP * TN], F32, tag="wsign")
            gtile = pp.tile([P, TP * DD], F32, tag="gtile")
            wtile = pp.tile([P, DS * BASE], F32, tag="wtile")
            absO2 = pp.tile([P, TP * DD * TN], F32, tag="absO2")
            onemg = pp.tile([P, TP * DD], F32, tag="onemg")
            wmag = pp.tile([P, TP * TN], F32, tag="wmag")
            signed = pp.tile([P, TP * TN], F32, tag="signed")
            logm = pp.tile([P, TP * TN], F32, tag="logm")
            denall = pp.tile([P, NCHUNK * GRP], F32, tag="denall")
            numall = pp.tile([P, NCHUNK * GRP], F32, tag="numall")

            nc.sync.dma_start(wtile[:], wpat)

            # ---- phase 1: digit softmax expected value -> vmag ----
            front_state = {}

            def p1_fd(ci):
                """DVE-side front: dma, group max, DVE share of subtract."""
                x = xp.tile([P, CW], F32, tag="x")
                nc.sync.dma_start(x[:], dl[:, ci * CW:(ci + 1) * CW])
                xv = x[:].rearrange("p (g b) -> p g b", b=BASE)
                m = sp.tile([P, GRP], F32, tag="m")
                nc.vector.tensor_reduce(m[:], xv, AX.X, ALU.max)
                front_state[ci] = (x, m)

            def p1_fp(ci):
                """Pool-side front: the full subtract (halves for earlier exp)."""
                x, m = front_state[ci]
                xv = x[:].rearrange("p (g b) -> p g b", b=BASE)
                d = dp.tile([P, CW], F32, tag="d")
                dv = d[:].rearrange("p (g b) -> p g b", b=BASE)
                mb = m[:].unsqueeze(2).broadcast_to((P, GRP, BASE))
                nc.gpsimd.tensor_tensor(dv[:, :SG], xv[:, :SG], mb[:, :SG], ALU.subtract)
                nc.gpsimd.tensor_tensor(dv[:, SG:], xv[:, SG:], mb[:, SG:], ALU.subtract)
                front_state[ci] = (x, m, d)

            def p1_e(ci):
                """exp halves; the DVE-subtract half (upper) is ready first."""
                x, m, d = front_state[ci]
                e = xp.tile([P, CW], F32, tag="x")
                HW = CW // 2
                nc.scalar.activation(e[:, :HW], d[:, :HW], ACTF.Exp, scale=INV_TEMP)
                nc.scalar.activation(e[:, HW:], d[:, HW:], ACTF.Exp, scale=INV_TEMP)
                front_state[ci] = e

            def p1_bp(ci):
                """weight-mult on DVE: keeps the num-reduce dependency engine-local."""
                e = front_state[ci]
                w = dp.tile([P, CW], F32, tag="d")
                wv = w[:].rearrange("p (r q) -> p r q", q=DS * BASE)
                ev8 = e[:].rearrange("p (r q) -> p r q", q=DS * BASE)
                wb = wtile[:].unsqueeze(1).broadcast_to((P, CH * NI, DS * BASE))
                nc.vector.tensor_tensor(wv[:, :HR], ev8[:, :HR], wb[:, :HR], ALU.mult)
                nc.vector.tensor_tensor(wv[:, HR:], ev8[:, HR:], wb[:, HR:], ALU.mult)
                front_state[ci] = (e, w)

            def p1_bd(ci):
                """DVE back: den/num reduces (early halves first) + reciprocal."""
                e, w = front_state.pop(ci)
                den = denall[:, ci * GRP:(ci + 1) * GRP]
                ev = e[:].rearrange("p (g b) -> p g b", b=BASE)
                HG = GRP // 2
                nc.vector.tensor_reduce(den[:, :HG], ev[:, :HG], AX.X, ALU.add)
                nc.vector.tensor_reduce(den[:, HG:], ev[:, HG:], AX.X, ALU.add)
                num = numall[:, ci * GRP:(ci + 1) * GRP]
                w3 = w[:].rearrange("p (g b) -> p g b", b=BASE)
                nc.vector.tensor_reduce(num[:, :HG], w3[:, :HG], AX.X, ALU.add)
                nc.vector.tensor_reduce(num[:, HG:], w3[:, HG:], AX.X, ALU.add)
                rcp = sp.tile([P, GRP], F32, tag="rcp")
                scr = sp.tile([P, GRP], F32, tag="scr")
                nc.vector.reciprocal_approx_accurate(rcp[:], den[:], scr[:])
                return rcp

            def p1_tail(ci, rcp):
                """expected value + pow-weighted sum (Pool) + clip (DVE)."""
                num = numall[:, ci * GRP:(ci + 1) * GRP]
                ex = sp.tile([P, GRP], F32, tag="ex")
                nc.gpsimd.tensor_tensor(ex[:], num, rcp[:], ALU.mult)
                ex3 = ex[:].rearrange("p (r d) -> p r d", d=DS)
                v4 = sp.tile([P, CH * NI * 4], F32, tag="v4")
                v43 = v4[:].rearrange("p (r d) -> p r d", d=4)
                nc.gpsimd.tensor_tensor(v43, ex3[:, :, 0:4], ex3[:, :, 4:8], ALU.add)
                v2 = sp.tile([P, CH * NI * 2], F32, tag="v2")
                v23 = v2[:].rearrange("p (r d) -> p r d", d=2)
                nc.gpsimd.tensor_tensor(v23, v43[:, :, 0:2], v43[:, :, 2:4], ALU.add)
                vm = sp.tile([P, CH * NI], F32, tag="vm")
                nc.gpsimd.tensor_tensor(vm[:], v23[:, :, 0], v23[:, :, 1], ALU.add)
                nc.vector.tensor_scalar(
                    vmag[:, ci * CH * NI:(ci + 1) * CH * NI], vm[:],
                    MAG_MIN, MAG_MAX, ALU.max, ALU.min,
                )

            p1_fd(0)
            p1_fp(0)
            p1_e(0)
            p1_fd(1)
            # phase-2 inputs after the first chunk DMAs so they don't block them
            nc.sync.dma_start(otile[:], od)
            nc.sync.dma_start(wsign[:], vsg)
            nc.sync.dma_start(gtile[:], gd)
            nc.scalar.activation(absO2[:], otile[:], ACTF.Abs, scale=2.0)
            nc.vector.tensor_scalar(onemg[:], gtile[:], -1.0, 1.0, ALU.mult, ALU.add)
            for ci in range(NCHUNK):
                p1_bp(ci)
                rcp = p1_bd(ci)
                if ci + 1 < NCHUNK:
                    p1_fp(ci + 1)
                    p1_e(ci + 1)
                if ci + 2 < NCHUNK:
                    p1_fd(ci + 2)
                p1_tail(ci, rcp)

            # ---- phase 2: DAG recurrence ----
            nc.gpsimd.memset(wmag[:], MAG_MIN)
            wm3 = wmag[:].rearrange("p (t n) -> p t n", n=TN)
            ws3 = wsign[:].rearrange("p (t n) -> p t n", n=TN)
            sg3 = signed[:].rearrange("p (t n) -> p t n", n=TN)
            lg3 = logm[:].rearrange("p (t n) -> p t n", n=TN)
            o4 = otile[:].rearrange("p (t s n) -> p t s n", s=DD, n=TN)
            a4 = absO2[:].rearrange("p (t s n) -> p t s n", s=DD, n=TN)

            nc.vector.tensor_copy(
                wm3[:, :, 0:NI], vmag[:].rearrange("p (t n) -> p t n", n=NI)
            )
            nc.vector.tensor_tensor(signed[:], wsign[:], wmag[:], ALU.mult)
            _emit_ln(nc, stp, logm[:], wmag[:], TP * TN, "lni")

            def gs(ap, s, n_bcast=None):
                v = ap[:, s::DD]
                if n_bcast is None:
                    return v
                return v.unsqueeze(2).broadcast_to((P, TP, n_bcast))

            def p2_front(s):
                """Heavy dots over nodes [0, K): everything but the node written
                by step s-1.  Depends only on >=2-step-old state, so it runs one
                step ahead of its consumer."""
                K = NI if s == 0 else NI - 1 + s
                t1 = stp.tile([P, TP * TN], F32, tag="t1f")
                t13 = t1[:].rearrange("p (t n) -> p t n", n=TN)
                nc.gpsimd.tensor_tensor(
                    t13[:, :, :K], lg3[:, :, :K], gs(onemg[:], s, K), ALU.mult
                )
                t2 = stp.tile([P, TP * TN], F32, tag="t2f")
                t23 = t2[:].rearrange("p (t n) -> p t n", n=TN)
                nc.vector.tensor_tensor(
                    t23[:, :, :K], sg3[:, :, :K], gs(gtile[:], s, K), ALU.mult
                )
                mx = stp.tile([P, TP * TN], F32, tag="mxf")
                mx3 = mx[:].rearrange("p (t n) -> p t n", n=TN)
                nc.vector.tensor_tensor(
                    mx3[:, :, :K], t13[:, :, :K], t23[:, :, :K], ALU.add
                )
                rt = stp.tile([P, TP * TN], F32, tag="rtf")
                rt3 = rt[:].rearrange("p (t n) -> p t n", n=TN)
                nc.vector.tensor_tensor(
                    rt3[:, :, :K], mx3[:, :, :K], o4[:, :, s, :K], ALU.mult
                )
                rold = st2.tile([P, TP], F32, tag="rold")
                nc.vector.tensor_reduce(rold[:], rt3[:, :, :K], AX.X, ALU.add)

                sw = stp.tile([P, TP * TN], F32, tag="swf")
                sw3 = sw[:].rearrange("p (t n) -> p t n", n=TN)
                nc.gpsimd.tensor_tensor(
                    sw3[:, :, :K], ws3[:, :, :K], a4[:, :, s, :K], ALU.mult
                )
                swp = stp.tile([P, TP * TN], F32, tag="swpf")
                swp3 = swp[:].rearrange("p (t n) -> p t n", n=TN)
                nc.scalar.activation(swp3[:, :, :K], sw3[:, :, :K], ACTF.Copy, bias=1.0)
                # product over the K nodes: pairwise multiply tree on Pool
                ta = st2.tile([P, TP * TN], F32, tag="ta")
                tb = st2.tile([P, TP * TN], F32, tag="tb")
                pbufs = [
                    ta[:].rearrange("p (t n) -> p t n", n=TN),
                    tb[:].rearrange("p (t n) -> p t n", n=TN),
                ]
                src3, width, pi = swp3, K, 0
                while width > 1:
                    half, odd = width // 2, width % 2
                    dst3 = pbufs[pi]
                    nc.gpsimd.tensor_tensor(
                        dst3[:, :, :half], src3[:, :, :half],
                        src3[:, :, half:2 * half], ALU.mult,
                    )
                    if odd:
                        nc.vector.tensor_copy(dst3[:, :, half], src3[:, :, 2 * half])
                    src3, width, pi = dst3, half + odd, 1 - pi
                return rold, src3[:, :, 0]

            res = sp.tile([P, TP], F32, tag="res")
            fr = p2_front(0)

            for s in range(DD):
                last = s == DD - 1
                rold, prodold = fr

                rp = st2.tile([P, 2 * TP], F32, tag="rp")  # [R | prod]
                if s == 0:
                    nc.vector.tensor_copy(rp[:, 0:TP], rold[:])
                    nc.vector.tensor_copy(rp[:, TP:2 * TP], prodold)
                else:
                    # fold in the node written by step s-1 (index NI-1+s)
                    nd = NI - 1 + s
                    q1 = stp.tile([P, TP], F32, tag="q1")
                    nc.vector.tensor_tensor(
                        q1[:], lg3[:, :, nd], gs(onemg[:], s), ALU.mult
                    )
                    q2 = stp.tile([P, TP], F32, tag="q2")
                    nc.vector.tensor_tensor(
                        q2[:], sg3[:, :, nd], gs(gtile[:], s), ALU.mult
                    )
                    mixn = stp.tile([P, TP], F32, tag="mixn")
                    nc.vector.tensor_tensor(mixn[:], q1[:], q2[:], ALU.add)
                    rn = stp.tile([P, TP], F32, tag="rn")
                    nc.vector.tensor_tensor(rn[:], mixn[:], o4[:, :, s, nd], ALU.mult)
                    nc.vector.tensor_tensor(rp[:, 0:TP], rold[:], rn[:], ALU.add)
                    swn = stp.tile([P, TP], F32, tag="swn")
                    nc.vector.tensor_tensor(swn[:], ws3[:, :, nd], a4[:, :, s, nd], ALU.mult)
                    nc.vector.tensor_scalar(swn[:], swn[:], 1.0, None, ALU.add)
                    nc.vector.tensor_tensor(rp[:, TP:2 * TP], prodold, swn[:], ALU.mult)

                # prefetch next step's heavy dots while this step's tail runs
                if not last:
                    fr = p2_front(s + 1)

                # tanh(y/SIGN_TEMP) = 1 - 2/(1+exp(2e4*y)) on [R | prod] at once
                yc = stp.tile([P, 2 * TP], F32, tag="yc")
                nc.vector.tensor_scalar(yc[:], rp[:], -0.005, 0.005, ALU.max, ALU.min)
                tE = stp.tile([P, 2 * TP], F32, tag="tE")
                nc.scalar.activation(tE[:], yc[:], ACTF.Exp, scale=2.0 * INV_SIGN_TEMP)
                u = stp.tile([P, 2 * TP], F32, tag="u")
                nc.vector.tensor_scalar(u[:], tE[:], 1.0, 1e30, ALU.add, ALU.min)
                rc2 = stp.tile([P, 2 * TP], F32, tag="rc2")
                sc2 = stp.tile([P, 2 * TP], F32, tag="sc2")
                nc.vector.reciprocal_approx_accurate(rc2[:], u[:], sc2[:])
                th = stp.tile([P, 2 * TP], F32, tag="th")
                nc.vector.tensor_scalar(th[:], rc2[:], -2.0, 1.0, ALU.mult, ALU.add)
                lin_sign, log_sign = th[:, 0:TP], th[:, TP:2 * TP]

                # s_new = G*lin + (1-G)*log  (clip dropped: convex combo of
                # values in [-1,1] stays within 1ulp of the range)
                sa = stp.tile([P, TP], F32, tag="sa")
                nc.vector.tensor_tensor(sa[:], lin_sign, gs(gtile[:], s), ALU.mult)
                sb = stp.tile([P, TP], F32, tag="sb")
                nc.vector.tensor_tensor(sb[:], log_sign, gs(onemg[:], s), ALU.mult)
                snew = stp.tile([P, TP], F32, tag="snew")
                nc.vector.tensor_tensor(snew[:], sa[:], sb[:], ALU.add)
                nc.vector.tensor_scalar(snew[:], snew[:], -1.0, 1.0, ALU.max, ALU.min)

                # m_new = G*min(|R|,MAX) + (1-G)*exp(clip(R,-100,EXP_CLAMP)), clipped.
                # The EXP_CLAMP upper bound replaces the reference's inf path: any
                # clamped value yields (1-G)*e^88.72 >= 2e31, which the final clip
                # maps to 1e28 exactly as inf would.
                R = rp[:, 0:TP]
                absR = stp.tile([P, TP], F32, tag="absR")
                nc.scalar.activation(absR[:], R, ACTF.Abs)
                nc.vector.tensor_scalar(absR[:], absR[:], MAG_MAX, None, ALU.min)
                rc = stp.tile([P, TP], F32, tag="rc")
                nc.vector.tensor_scalar(rc[:], R, -LOG_LIM, EXP_CLAMP, ALU.max, ALU.min)
                logres = stp.tile([P, TP], F32, tag="logres")
                nc.scalar.activation(logres[:], rc[:], ACTF.Exp)
                ma = stp.tile([P, TP], F32, tag="ma")
                nc.vector.tensor_tensor(ma[:], absR[:], gs(gtile[:], s), ALU.mult)
                mb2 = stp.tile([P, TP], F32, tag="mb2")
                nc.vector.tensor_tensor(mb2[:], logres[:], gs(onemg[:], s), ALU.mult)
                mnew = stp.tile([P, TP], F32, tag="mnew")
                nc.vector.tensor_tensor(mnew[:], ma[:], mb2[:], ALU.add)
                nc.vector.tensor_scalar(mnew[:], mnew[:], MAG_MIN, MAG_MAX, ALU.max, ALU.min)

                if last:
                    # output = wsign[16] * wmag[16]; skip the state writes
                    nc.vector.tensor_tensor(res[:], mnew[:], snew[:], ALU.mult)
                else:
                    ni = NI + s
                    nc.vector.tensor_copy(wm3[:, :, ni], mnew[:])
                    nc.vector.tensor_copy(ws3[:, :, ni], snew[:])
                    nc.gpsimd.tensor_tensor(sg3[:, :, ni], mnew[:], snew[:], ALU.mult)
                    _emit_ln(nc, stp, lg3[:, :, ni], mnew[:], TP, "lns")

            nc.sync.dma_start(out, res[:])

    nc.compile()
    return nc


_NC_CACHE = None


def _get_nc():
    global _NC_CACHE
    if _NC_CACHE is None:
        _NC_CACHE = build_program()
    return _NC_CACHE


def make_in_maps(digit_logits, V_sign, O, G):
    dlf = np.ascontiguousarray(digit_logits, dtype=np.float32).reshape(B * T, PTOK)
    vsf = np.ascontiguousarray(V_sign, dtype=np.float32).reshape(B * T, TN)
    of = np.ascontiguousarray(O, dtype=np.float32).reshape(B * T, DD * TN)
    gf = np.ascontiguousarray(G, dtype=np.float32).reshape(B * T, DD)
    pat = np.zeros(DS * BASE, dtype=np.float32)
    for dd in range(DS):
        for i in range(BASE):
            pat[dd * BASE + i] = i * POWERS[dd]
    wpat = np.tile(pat[None, :], (P, 1))
    in_maps = []
    for c in range(NCORES):
        s0, s1 = c * TOK_PER_CORE, (c + 1) * TOK_PER_CORE
        in_maps.append({
            "dl": np.ascontiguousarray(dlf[s0:s1].reshape(P, TP * PTOK)),
            "vsg": np.ascontiguousarray(vsf[s0:s1].reshape(P, TP * TN)),
            "od": np.ascontiguousarray(of[s0:s1].reshape(P, TP * DD * TN)),
            "gd": np.ascontiguousarray(gf[s0:s1].reshape(P, TP * DD)),
            "wpat": wpat,
        })
    return in_maps


def kernel(digit_logits, V_sign, O, G, _trace=False, _return_results=False):
    nc = _get_nc()
    in_maps = make_in_maps(digit_logits, V_sign, O, G)
    res = run_bass_kernel_spmd(nc, in_maps, list(range(NCORES)), trace=_trace)
    outs = [np.asarray(res.results[c]["out"]).reshape(TOK_PER_CORE) for c in range(NCORES)]
    full = np.concatenate(outs).reshape(B, T)
    if _return_results:
        return full, res
    return full


# revision 42
# speedup vs baseline: 1.0855x; 1.0015x over previous
"""Trainium2 Bass kernel for nn_DAGExecutor (digit-softmax + 8-step DAG recurrence).

Fully data-parallel: 32768 (B*T) tokens sharded as 4096 tokens per core across
8 cores; no cross-core communication.  Per core tokens live as [128 partitions
x 32 tokens].

Engine split: ACT does exp/ln/abs; Pool (gpsimd) takes ~90% of the two big
phase-1 elementwise passes plus assorted multiplies; DVE owns the grouped
reduces and the small-op chains.  Phase 1 is software-pipelined (front/back
skew) because engine queues execute in program order.  Phase 2 pipelines each
step's heavy node-dots ("front", which only need state that is two steps old)
one step ahead, leaving a short per-token fixup chain on the critical path.
"""

import sys

for _p in ("/opt/trn_rl_repo",):
    if _p not in sys.path:
        sys.path.insert(0, _p)

import numpy as np

import concourse.bass as bass
import concourse.bacc as bacc
import concourse.mybir as mybir
import concourse.tile as tile
from concourse.bass_utils import run_bass_kernel_spmd

F32 = mybir.dt.float32
I32 = mybir.dt.int32
AX = mybir.AxisListType
ALU = mybir.AluOpType
ACTF = mybir.ActivationFunctionType

# ---- problem constants (hardcoded; must match reference setup) ----
B, T = 8, 4096
NI = 9          # initial nodes
DD = 8          # dag depth
TN = 17         # total nodes
DS = 8          # digit slots
BASE = 10
PTOK = 720      # values per token in digit_logits = NI*DS*BASE
NCORES = 8
TOK_PER_CORE = (B * T) // NCORES        # 4096
P = 128                                  # partitions
TP = TOK_PER_CORE // P                   # 32 tokens per partition
MAG_MIN, MAG_MAX = 1e-12, 1e28
LOG_LIM = 100.0
INV_TEMP = 100.0                         # 1/TEMP
INV_SIGN_TEMP = 1e4                      # 1/SIGN_TEMP
# f32 exp overflow boundary: clamping exp's input here yields a value big
# enough that the downstream [1e-12,1e28] clip matches the reference's inf path
EXP_CLAMP = 88.7228355
POWERS = np.asarray([float(BASE) ** (4 - 1 - d) for d in range(DS)], dtype=np.float32)
LN2 = float(np.log(np.float32(2.0)))

CH = 4                                   # tokens-per-partition per phase-1 chunk
NCHUNK = TP // CH                        # 8 chunks
GRP = CH * NI * DS                       # softmax groups per chunk (288)
CW = CH * PTOK                           # chunk width (2880)
SG = GRP // 2                            # Pool share of the subtract pass
HR = (CH * NI) // 2                      # half of the token*node axis


def _emit_ln(nc, pool, dst_ap, src_ap, width, tag):
    """dst = ln(src) for src in [1e-12, +inf] via exponent/mantissa split.

    ACT Ln only covers |x| <= 2^64 and direct rescaling costs accuracy near
    ln(x)=0; the split keeps ~2ulp everywhere."""
    xb = src_ap.bitcast(I32)
    eint = pool.tile([P, width], I32, tag=tag + "_e")
    nc.vector.tensor_scalar(eint[:], xb, 23, None, ALU.logical_shift_right)
    le = pool.tile([P, width], F32, tag=tag + "_le")
    nc.vector.tensor_scalar(le[:], eint[:], 127.0, LN2, ALU.subtract, ALU.mult)
    mbits = pool.tile([P, width], I32, tag=tag + "_mb")
    nc.vector.tensor_scalar(
        mbits[:], xb, 0x007FFFFF, 0x3F800000, ALU.bitwise_and, ALU.bitwise_or
    )
    lnm = pool.tile([P, width], F32, tag=tag + "_lm")
    nc.scalar.activation(lnm[:], mbits[:].bitcast(F32), ACTF.Ln)
    nc.vector.tensor_tensor(dst_ap, lnm[:], le[:], ALU.add)


def _patch_act_tables():
    """Force all activations onto the natural_log_exp_and_others table set.

    The table-load pass greedily alternates exp_and_others / natural_log,
    inserting ~18 ACT table loads (~2.7us each on HW).  Emptying every other
    set (indices preserved) makes the combined set the only candidate."""
    import concourse.hw_specs as hw_specs
    orig = hw_specs.get_activation_tables

    def patched(arch):
        tabs = orig(arch)
        keep = "natural_log_exp_and_others"
        if keep not in tabs:
            return tabs
        return {k: (v if k == keep else set()) for k, v in tabs.items()}

    patched.__wrapped__ = orig
    bacc.get_activation_tables = patched


def build_program():
    _patch_act_tables()
    nc = bacc.Bacc("TRN2", target_bir_lowering=False, debug=False)

    dl = nc.dram_tensor("dl", [P, TP * PTOK], F32, kind="ExternalInput").ap()
    vsg = nc.dram_tensor("vsg", [P, TP * TN], F32, kind="ExternalInput").ap()
    od = nc.dram_tensor("od", [P, TP * DD * TN], F32, kind="ExternalInput").ap()
    gd = nc.dram_tensor("gd", [P, TP * DD], F32, kind="ExternalInput").ap()
    wpat = nc.dram_tensor("wpat", [P, DS * BASE], F32, kind="ExternalInput").ap()
    out = nc.dram_tensor("out", [P, TP], F32, kind="ExternalOutput").ap()

    with tile.TileContext(nc) as tc:
        with (
            tc.tile_pool(name="persist", bufs=1) as pp,
            tc.tile_pool(name="xin", bufs=4) as xp,
            tc.tile_pool(name="dbuf", bufs=3) as dp,
            tc.tile_pool(name="small", bufs=3) as sp,
            tc.tile_pool(name="steps", bufs=1) as stp,
            tc.tile_pool(name="steps2", bufs=2) as st2,
        ):
            # ---- persistent tiles ----
            vmag = pp.tile([P, TP * NI], F32, tag="vmag")
            otile = pp.tile([P, TP * DD * TN], F32, tag="otile")
            wsign = pp.tile([P, TP * TN], F32, tag="wsign")
            gtile = pp.tile([P, TP * DD], F32, tag="gtile")
            wtile = pp.tile([P, DS * BASE], F32, tag="wtile")
            absO2 = pp.tile([P, TP * DD * TN], F32, tag="absO2")
            onemg = pp.tile([P, TP * DD], F32, tag="onemg")
            wmag = pp.tile([P, TP * TN], F32, tag="wmag")
            signed = pp.tile([P, TP * TN], F32, tag="signed")
            logm = pp.tile([P, TP * TN], F32, tag="logm")
            denall = pp.tile([P, NCHUNK * GRP], F32, tag="denall")
            numall = pp.tile([P, NCHUNK * GRP], F32, tag="numall")

            nc.sync.dma_start(wtile[:], wpat)
            nc.gpsimd.memset(wmag[:], MAG_MIN)

            # ---- phase 1: digit softmax expected value -> vmag ----
            front_state = {}

            def p1_fd(ci):
                """DVE-side front: dma, group max, DVE share of subtract."""
                x = xp.tile([P, CW], F32, tag="x")
                nc.sync.dma_start(x[:], dl[:, ci * CW:(ci + 1) * CW])
                xv = x[:].rearrange("p (g b) -> p g b", b=BASE)
                m = sp.tile([P, GRP], F32, tag="m")
                nc.vector.tensor_reduce(m[:], xv, AX.X, ALU.max)
                front_state[ci] = (x, m)

            def p1_fp(ci):
                """Pool-side front: the full subtract (halves for earlier exp)."""
                x, m = front_state[ci]
                xv = x[:].rearrange("p (g b) -> p g b", b=BASE)
                d = dp.tile([P, CW], F32, tag="d")
                dv = d[:].rearrange("p (g b) -> p g b", b=BASE)
                mb = m[:].unsqueeze(2).broadcast_to((P, GRP, BASE))
                nc.gpsimd.tensor_tensor(dv[:, :SG], xv[:, :SG], mb[:, :SG], ALU.subtract)
                nc.gpsimd.tensor_tensor(dv[:, SG:], xv[:, SG:], mb[:, SG:], ALU.subtract)
                front_state[ci] = (x, m, d)

            def p1_e(ci):
                """exp halves; the DVE-subtract half (upper) is ready first."""
                x, m, d = front_state[ci]
                e = xp.tile([P, CW], F32, tag="x")
                HW = CW // 2
                nc.scalar.activation(e[:, :HW], d[:, :HW], ACTF.Exp, scale=INV_TEMP)
                nc.scalar.activation(e[:, HW:], d[:, HW:], ACTF.Exp, scale=INV_TEMP)
                front_state[ci] = e

            def p1_bp(ci):
                """weight-mult on DVE: keeps the num-reduce dependency engine-local."""
                e = front_state[ci]
                w = dp.tile([P, CW], F32, tag="d")
                wv = w[:].rearrange("p (r q) -> p r q", q=DS * BASE)
                ev8 = e[:].rearrange("p (r q) -> p r q", q=DS * BASE)
                wb = wtile[:].unsqueeze(1).broadcast_to((P, CH * NI, DS * BASE))
                nc.vector.tensor_tensor(wv[:, :HR], ev8[:, :HR], wb[:, :HR], ALU.mult)
                nc.vector.tensor_tensor(wv[:, HR:], ev8[:, HR:], wb[:, HR:], ALU.mult)
                front_state[ci] = (e, w)

            def p1_bd(ci):
                """DVE back: den/num reduces (early halves first) + reciprocal."""
                e, w = front_state.pop(ci)
                den = denall[:, ci * GRP:(ci + 1) * GRP]
                ev = e[:].rearrange("p (g b) -> p g b", b=BASE)
                HG = GRP // 2
                nc.vector.tensor_reduce(den[:, :HG], ev[:, :HG], AX.X, ALU.add)
                nc.vector.tensor_reduce(den[:, HG:], ev[:, HG:], AX.X, ALU.add)
                num = numall[:, ci * GRP:(ci + 1) * GRP]
                w3 = w[:].rearrange("p (g b) -> p g b", b=BASE)
                nc.vector.tensor_reduce(num[:, :HG], w3[:, :HG], AX.X, ALU.add)
                nc.vector.tensor_reduce(num[:, HG:], w3[:, HG:], AX.X, ALU.add)
                rcp = sp.tile([P, GRP], F32, tag="rcp")
                scr = sp.tile([P, GRP], F32, tag="scr")
                nc.vector.reciprocal_approx_accurate(rcp[:], den[:], scr[:])
                return rcp

            def p1_tail(ci, rcp):
                """expected value + pow-weighted sum (Pool) + clip (DVE)."""
                num = numall[:, ci * GRP:(ci + 1) * GRP]
                ex = sp.tile([P, GRP], F32, tag="ex")
                nc.gpsimd.tensor_tensor(ex[:], num, rcp[:], ALU.mult)
                ex3 = ex[:].rearrange("p (r d) -> p r d", d=DS)
                v4 = sp.tile([P, CH * NI * 4], F32, tag="v4")
                v43 = v4[:].rearrange("p (r d) -> p r d", d=4)
                nc.gpsimd.tensor_tensor(v43, ex3[:, :, 0:4], ex3[:, :, 4:8], ALU.add)
                v2 = sp.tile([P, CH * NI * 2], F32, tag="v2")
                v23 = v2[:].rearrange("p (r d) -> p r d", d=2)
                nc.gpsimd.tensor_tensor(v23, v43[:, :, 0:2], v43[:, :, 2:4], ALU.add)
                vm = sp.tile([P, CH * NI], F32, tag="vm")
                nc.gpsimd.tensor_tensor(vm[:], v23[:, :, 0], v23[:, :, 1], ALU.add)
                nc.vector.tensor_scalar(
                    vmag[:, ci * CH * NI:(ci + 1) * CH * NI], vm[:],
                    MAG_MIN, MAG_MAX, ALU.max, ALU.min,
                )

            p1_fd(0)
            p1_fp(0)
            p1_e(0)
            p1_fd(1)
            # phase-2 inputs after the first chunk DMAs so they don't block them
            nc.sync.dma_start(otile[:], od)
            nc.sync.dma_start(wsign[:], vsg)
            nc.sync.dma_start(gtile[:], gd)
            nc.scalar.activation(absO2[:], otile[:], ACTF.Abs, scale=2.0)
            nc.vector.tensor_scalar(onemg[:], gtile[:], -1.0, 1.0, ALU.mult, ALU.add)
            for ci in range(NCHUNK):
                p1_bp(ci)
                rcp = p1_bd(ci)
                if ci + 1 < NCHUNK:
                    p1_fp(ci + 1)
                    p1_e(ci + 1)
                if ci + 2 < NCHUNK:
                    p1_fd(ci + 2)
                p1_tail(ci, rcp)

            # ---- phase 2: DAG recurrence ----
            wm3 = wmag[:].rearrange("p (t n) -> p t n", n=TN)
            nc.vector.tensor_copy(
                wm3[:, :, 0:NI], vmag[:].rearrange("p (t n) -> p t n", n=NI)
            )
            nc.vector.tensor_tensor(signed[:], wsign[:], wmag[:], ALU.mult)
            _emit_ln(nc, stp, logm[:], wmag[:], TP * TN, "lni")
            ws3 = wsign[:].rearrange("p (t n) -> p t n", n=TN)
            sg3 = signed[:].rearrange("p (t n) -> p t n", n=TN)
            lg3 = logm[:].rearrange("p (t n) -> p t n", n=TN)
            o4 = otile[:].rearrange("p (t s n) -> p t s n", s=DD, n=TN)
            a4 = absO2[:].rearrange("p (t s n) -> p t s n", s=DD, n=TN)


            def gs(ap, s, n_bcast=None):
                v = ap[:, s::DD]
                if n_bcast is None:
                    return v
                return v.unsqueeze(2).broadcast_to((P, TP, n_bcast))

            def p2_front(s):
                """Heavy dots over nodes [0, K): everything but the node written
                by step s-1.  Depends only on >=2-step-old state, so it runs one
                step ahead of its consumer."""
                K = NI if s == 0 else NI - 1 + s
                t1 = stp.tile([P, TP * TN], F32, tag="t1f")
                t13 = t1[:].rearrange("p (t n) -> p t n", n=TN)
                nc.gpsimd.tensor_tensor(
                    t13[:, :, :K], lg3[:, :, :K], gs(onemg[:], s, K), ALU.mult
                )
                t2 = stp.tile([P, TP * TN], F32, tag="t2f")
                t23 = t2[:].rearrange("p (t n) -> p t n", n=TN)
                nc.vector.tensor_tensor(
                    t23[:, :, :K], sg3[:, :, :K], gs(gtile[:], s, K), ALU.mult
                )
                mx = stp.tile([P, TP * TN], F32, tag="mxf")
                mx3 = mx[:].rearrange("p (t n) -> p t n", n=TN)
                nc.vector.tensor_tensor(
                    mx3[:, :, :K], t13[:, :, :K], t23[:, :, :K], ALU.add
                )
                rt = stp.tile([P, TP * TN], F32, tag="rtf")
                rt3 = rt[:].rearrange("p (t n) -> p t n", n=TN)
                nc.vector.tensor_tensor(
                    rt3[:, :, :K], mx3[:, :, :K], o4[:, :, s, :K], ALU.mult
                )
                rold = st2.tile([P, TP], F32, tag="rold")
                nc.vector.tensor_reduce(rold[:], rt3[:, :, :K], AX.X, ALU.add)

                sw = stp.tile([P, TP * TN], F32, tag="swf")
                sw3 = sw[:].rearrange("p (t n) -> p t n", n=TN)
                nc.gpsimd.tensor_tensor(
                    sw3[:, :, :K], ws3[:, :, :K], a4[:, :, s, :K], ALU.mult
                )
                swp = stp.tile([P, TP * TN], F32, tag="swpf")
                swp3 = swp[:].rearrange("p (t n) -> p t n", n=TN)
                nc.scalar.activation(swp3[:, :, :K], sw3[:, :, :K], ACTF.Copy, bias=1.0)
                # product over the K nodes: pairwise multiply tree on Pool
                ta = st2.tile([P, TP * TN], F32, tag="ta")
                tb = st2.tile([P, TP * TN], F32, tag="tb")
                pbufs = [
                    ta[:].rearrange("p (t n) -> p t n", n=TN),
                    tb[:].rearrange("p (t n) -> p t n", n=TN),
                ]
                src3, width, pi = swp3, K, 0
                while width > 1:
                    half, odd = width // 2, width % 2
                    dst3 = pbufs[pi]
                    nc.gpsimd.tensor_tensor(
                        dst3[:, :, :half], src3[:, :, :half],
                        src3[:, :, half:2 * half], ALU.mult,
                    )
                    if odd:
                        nc.vector.tensor_copy(dst3[:, :, half], src3[:, :, 2 * half])
                    src3, width, pi = dst3, half + odd, 1 - pi
                return rold, src3[:, :, 0]

            res = sp.tile([P, TP], F32, tag="res")
            fr = p2_front(0)

            for s in range(DD):
                last = s == DD - 1
                rold, prodold = fr

                rp = st2.tile([P, 2 * TP], F32, tag="rp")  # [R | prod]
                if s == 0:
                    nc.vector.tensor_copy(rp[:, 0:TP], rold[:])
                    nc.vector.tensor_copy(rp[:, TP:2 * TP], prodold)
                else:
                    # fold in the node written by step s-1 (index NI-1+s)
                    nd = NI - 1 + s
                    q1 = stp.tile([P, TP], F32, tag="q1")
                    nc.vector.tensor_tensor(
                        q1[:], lg3[:, :, nd], gs(onemg[:], s), ALU.mult
                    )
                    q2 = stp.tile([P, TP], F32, tag="q2")
                    nc.vector.tensor_tensor(
                        q2[:], sg3[:, :, nd], gs(gtile[:], s), ALU.mult
                    )
                    mixn = stp.tile([P, TP], F32, tag="mixn")
                    nc.vector.tensor_tensor(mixn[:], q1[:], q2[:], ALU.add)
                    rn = stp.tile([P, TP], F32, tag="rn")
                    nc.vector.tensor_tensor(rn[:], mixn[:], o4[:, :, s, nd], ALU.mult)
                    nc.vector.tensor_tensor(rp[:, 0:TP], rold[:], rn[:], ALU.add)
                    swn = stp.tile([P, TP], F32, tag="swn")
                    nc.vector.tensor_tensor(swn[:], ws3[:, :, nd], a4[:, :, s, nd], ALU.mult)
                    nc.vector.tensor_scalar(swn[:], swn[:], 1.0, None, ALU.add)
                    nc.vector.tensor_tensor(rp[:, TP:2 * TP], prodold, swn[:], ALU.mult)

                # prefetch next step's heavy dots while this step's tail runs
                if not last:
                    fr = p2_front(s + 1)

                # tanh(y/SIGN_TEMP) = 1 - 2/(1+exp(2e4*y)) on [R | prod] at once
                yc = stp.tile([P, 2 * TP], F32, tag="yc")
                nc.vector.tensor_scalar(yc[:], rp[:], -0.005, 0.005, ALU.max, ALU.min)
                tE = stp.tile([P, 2 * TP], F32, tag="tE")
                nc.scalar.activation(tE[:], yc[:], ACTF.Exp, scale=2.0 * INV_SIGN_TEMP)
                u = stp.tile([P, 2 * TP], F32, tag="u")
                nc.vector.tensor_scalar(u[:], tE[:], 1.0, 1e30, ALU.add, ALU.min)
                rc2 = stp.tile([P, 2 * TP], F32, tag="rc2")
                sc2 = stp.tile([P, 2 * TP], F32, tag="sc2")
                nc.vector.reciprocal_approx_accurate(rc2[:], u[:], sc2[:])
                th = stp.tile([P, 2 * TP], F32, tag="th")
                nc.vector.tensor_scalar(th[:], rc2[:], -2.0, 1.0, ALU.mult, ALU.add)
                lin_sign, log_sign = th[:, 0:TP], th[:, TP:2 * TP]

                # s_new = G*lin + (1-G)*log  (clip dropped: convex combo of
                # values in [-1,1] stays within 1ulp of the range)
                sa = stp.tile([P, TP], F32, tag="sa")
                nc.vector.tensor_tensor(sa[:], lin_sign, gs(gtile[:], s), ALU.mult)
                sb = stp.tile([P, TP], F32, tag="sb")
                nc.vector.tensor_tensor(sb[:], log_sign, gs(onemg[:], s), ALU.mult)
                snew = stp.tile([P, TP], F32, tag="snew")
                nc.vector.tensor_tensor(snew[:], sa[:], sb[:], ALU.add)
                nc.vector.tensor_scalar(snew[:], snew[:], -1.0, 1.0, ALU.max, ALU.min)

                # m_new = G*min(|R|,MAX) + (1-G)*exp(clip(R,-100,EXP_CLAMP)), clipped.
                # The EXP_CLAMP upper bound replaces the reference's inf path: any
                # clamped value yields (1-G)*e^88.72 >= 2e31, which the final clip
                # maps to 1e28 exactly as inf would.
                R = rp[:, 0:TP]
                absR = stp.tile([P, TP], F32, tag="absR")
                nc.scalar.activation(absR[:], R, ACTF.Abs)
                nc.vector.tensor_scalar(absR[:], absR[:], MAG_MAX, None, ALU.min)
                rc = stp.tile([P, TP], F32, tag="rc")
                nc.vector.tensor_scalar(rc[:], R, -LOG_LIM, EXP_CLAMP, ALU.max, ALU.min)
                logres = stp.tile([P, TP], F32, tag="logres")
                nc.scalar.activation(logres[:], rc[:], ACTF.Exp)
                ma = stp.tile([P, TP], F32, tag="ma")
                nc.vector.tensor_tensor(ma[:], absR[:], gs(gtile[:], s), ALU.mult)
                mb2 = stp.tile([P, TP], F32, tag="mb2")
                nc.vector.tensor_tensor(mb2[:], logres[:], gs(onemg[:], s), ALU.mult)
                mnew = stp.tile([P, TP], F32, tag="mnew")
                nc.vector.tensor_tensor(mnew[:], ma[:], mb2[:], ALU.add)
                nc.vector.tensor_scalar(mnew[:], mnew[:], MAG_MIN, MAG_MAX, ALU.max, ALU.min)

                if last:
                    # output = wsign[16] * wmag[16]; skip the state writes
                    nc.vector.tensor_tensor(res[:], mnew[:], snew[:], ALU.mult)
                else:
                    ni = NI + s
                    nc.vector.tensor_copy(wm3[:, :, ni], mnew[:])
                    nc.vector.tensor_copy(ws3[:, :, ni], snew[:])
                    nc.gpsimd.tensor_tensor(sg3[:, :, ni], mnew[:], snew[:], ALU.mult)
                    _emit_ln(nc, stp, lg3[:, :, ni], mnew[:], TP, "lns")

            nc.sync.dma_start(out, res[:])

    nc.compile()
    return nc


_NC_CACHE = None


def _get_nc():
    global _NC_CACHE
    if _NC_CACHE is None:
        _NC_CACHE = build_program()
    return _NC_CACHE


def make_in_maps(digit_logits, V_sign, O, G):
    dlf = np.ascontiguousarray(digit_logits, dtype=np.float32).reshape(B * T, PTOK)
    vsf = np.ascontiguousarray(V_sign, dtype=np.float32).reshape(B * T, TN)
    of = np.ascontiguousarray(O, dtype=np.float32).reshape(B * T, DD * TN)
    gf = np.ascontiguousarray(G, dtype=np.float32).reshape(B * T, DD)
    pat = np.zeros(DS * BASE, dtype=np.float32)
    for dd in range(DS):
        for i in range(BASE):
            pat[dd * BASE + i] = i * POWERS[dd]
    wpat = np.tile(pat[None, :], (P, 1))
    in_maps = []
    for c in range(NCORES):
        s0, s1 = c * TOK_PER_CORE, (c + 1) * TOK_PER_CORE
        in_maps.append({
            "dl": np.ascontiguousarray(dlf[s0:s1].reshape(P, TP * PTOK)),
            "vsg": np.ascontiguousarray(vsf[s0:s1].reshape(P, TP * TN)),
            "od": np.ascontiguousarray(of[s0:s1].reshape(P, TP * DD * TN)),
            "gd": np.ascontiguousarray(gf[s0:s1].reshape(P, TP * DD)),
            "wpat": wpat,
        })
    return in_maps


def kernel(digit_logits, V_sign, O, G, _trace=False, _return_results=False):
    nc = _get_nc()
    in_maps = make_in_maps(digit_logits, V_sign, O, G)
    res = run_bass_kernel_spmd(nc, in_maps, list(range(NCORES)), trace=_trace)
    outs = [np.asarray(res.results[c]["out"]).reshape(TOK_PER_CORE) for c in range(NCORES)]
    full = np.concatenate(outs).reshape(B, T)
    if _return_results:
        return full, res
    return full


# revision 43
# speedup vs baseline: 1.0987x; 1.0122x over previous
"""Trainium2 Bass kernel for nn_DAGExecutor (digit-softmax + 8-step DAG recurrence).

Fully data-parallel: 32768 (B*T) tokens sharded as 4096 tokens per core across
8 cores; no cross-core communication.  Per core tokens live as [128 partitions
x 32 tokens].

Engine split: ACT does exp/ln/abs; Pool (gpsimd) takes ~90% of the two big
phase-1 elementwise passes plus assorted multiplies; DVE owns the grouped
reduces and the small-op chains.  Phase 1 is software-pipelined (front/back
skew) because engine queues execute in program order.  Phase 2 pipelines each
step's heavy node-dots ("front", which only need state that is two steps old)
one step ahead, leaving a short per-token fixup chain on the critical path.
"""

import sys

for _p in ("/opt/trn_rl_repo",):
    if _p not in sys.path:
        sys.path.insert(0, _p)

import numpy as np

import concourse.bass as bass
import concourse.bacc as bacc
import concourse.mybir as mybir
import concourse.tile as tile
from concourse.bass_utils import run_bass_kernel_spmd

F32 = mybir.dt.float32
I32 = mybir.dt.int32
AX = mybir.AxisListType
ALU = mybir.AluOpType
ACTF = mybir.ActivationFunctionType

# ---- problem constants (hardcoded; must match reference setup) ----
B, T = 8, 4096
NI = 9          # initial nodes
DD = 8          # dag depth
TN = 17         # total nodes
DS = 8          # digit slots
BASE = 10
PTOK = 720      # values per token in digit_logits = NI*DS*BASE
NCORES = 8
TOK_PER_CORE = (B * T) // NCORES        # 4096
P = 128                                  # partitions
TP = TOK_PER_CORE // P                   # 32 tokens per partition
MAG_MIN, MAG_MAX = 1e-12, 1e28
LOG_LIM = 100.0
INV_TEMP = 100.0                         # 1/TEMP
INV_SIGN_TEMP = 1e4                      # 1/SIGN_TEMP
# f32 exp overflow boundary: clamping exp's input here yields a value big
# enough that the downstream [1e-12,1e28] clip matches the reference's inf path
EXP_CLAMP = 88.7228355
POWERS = np.asarray([float(BASE) ** (4 - 1 - d) for d in range(DS)], dtype=np.float32)
LN2 = float(np.log(np.float32(2.0)))

CH = 4                                   # tokens-per-partition per phase-1 chunk
NCHUNK = TP // CH                        # 8 chunks
GRP = CH * NI * DS                       # softmax groups per chunk (288)
CW = CH * PTOK                           # chunk width (2880)
SG = GRP // 2                            # Pool share of the subtract pass
HR = (CH * NI) // 2                      # half of the token*node axis


def _emit_ln(nc, pool, dst_ap, src_ap, width, tag):
    """dst = ln(src) for src in [1e-12, +inf] via exponent/mantissa split.

    ACT Ln only covers |x| <= 2^64 and direct rescaling costs accuracy near
    ln(x)=0; the split keeps ~2ulp everywhere."""
    xb = src_ap.bitcast(I32)
    eint = pool.tile([P, width], I32, tag=tag + "_e")
    nc.vector.tensor_scalar(eint[:], xb, 23, None, ALU.logical_shift_right)
    le = pool.tile([P, width], F32, tag=tag + "_le")
    nc.vector.tensor_scalar(le[:], eint[:], 127.0, LN2, ALU.subtract, ALU.mult)
    mbits = pool.tile([P, width], I32, tag=tag + "_mb")
    nc.vector.tensor_scalar(
        mbits[:], xb, 0x007FFFFF, 0x3F800000, ALU.bitwise_and, ALU.bitwise_or
    )
    lnm = pool.tile([P, width], F32, tag=tag + "_lm")
    nc.scalar.activation(lnm[:], mbits[:].bitcast(F32), ACTF.Ln)
    nc.vector.tensor_tensor(dst_ap, lnm[:], le[:], ALU.add)


def _patch_act_tables():
    """Force all activations onto the natural_log_exp_and_others table set.

    The table-load pass greedily alternates exp_and_others / natural_log,
    inserting ~18 ACT table loads (~2.7us each on HW).  Emptying every other
    set (indices preserved) makes the combined set the only candidate."""
    import concourse.hw_specs as hw_specs
    orig = hw_specs.get_activation_tables

    def patched(arch):
        tabs = orig(arch)
        keep = "natural_log_exp_and_others"
        if keep not in tabs:
            return tabs
        return {k: (v if k == keep else set()) for k, v in tabs.items()}

    patched.__wrapped__ = orig
    bacc.get_activation_tables = patched


def build_program():
    _patch_act_tables()
    nc = bacc.Bacc("TRN2", target_bir_lowering=False, debug=False)

    dl = nc.dram_tensor("dl", [P, TP * PTOK], F32, kind="ExternalInput").ap()
    vsg = nc.dram_tensor("vsg", [P, TP * TN], F32, kind="ExternalInput").ap()
    od = nc.dram_tensor("od", [P, TP * DD * TN], F32, kind="ExternalInput").ap()
    gd = nc.dram_tensor("gd", [P, TP * DD], F32, kind="ExternalInput").ap()
    wpat = nc.dram_tensor("wpat", [P, DS * BASE], F32, kind="ExternalInput").ap()
    out = nc.dram_tensor("out", [P, TP], F32, kind="ExternalOutput").ap()

    with tile.TileContext(nc) as tc:
        with (
            tc.tile_pool(name="persist", bufs=1) as pp,
            tc.tile_pool(name="xin", bufs=4) as xp,
            tc.tile_pool(name="dbuf", bufs=3) as dp,
            tc.tile_pool(name="small", bufs=3) as sp,
            tc.tile_pool(name="steps", bufs=1) as stp,
            tc.tile_pool(name="steps2", bufs=2) as st2,
        ):
            # ---- persistent tiles ----
            vmag = pp.tile([P, TP * NI], F32, tag="vmag")
            otile = pp.tile([P, TP * DD * TN], F32, tag="otile")
            wsign = pp.tile([P, TP * TN], F32, tag="wsign")
            gtile = pp.tile([P, TP * DD], F32, tag="gtile")
            wtile = pp.tile([P, DS * BASE], F32, tag="wtile")
            absO2 = pp.tile([P, TP * DD * TN], F32, tag="absO2")
            onemg = pp.tile([P, TP * DD], F32, tag="onemg")
            wmag = pp.tile([P, TP * TN], F32, tag="wmag")
            signed = pp.tile([P, TP * TN], F32, tag="signed")
            logm = pp.tile([P, TP * TN], F32, tag="logm")
            diffc = pp.tile([P, TP * TN], F32, tag="diffc")
            denall = pp.tile([P, NCHUNK * GRP], F32, tag="denall")
            numall = pp.tile([P, NCHUNK * GRP], F32, tag="numall")

            nc.sync.dma_start(wtile[:], wpat)
            nc.gpsimd.memset(wmag[:], MAG_MIN)

            # ---- phase 1: digit softmax expected value -> vmag ----
            front_state = {}

            def p1_fd(ci):
                """DVE-side front: dma, group max, DVE share of subtract."""
                x = xp.tile([P, CW], F32, tag="x")
                nc.sync.dma_start(x[:], dl[:, ci * CW:(ci + 1) * CW])
                xv = x[:].rearrange("p (g b) -> p g b", b=BASE)
                m = sp.tile([P, GRP], F32, tag="m")
                nc.vector.tensor_reduce(m[:], xv, AX.X, ALU.max)
                front_state[ci] = (x, m)

            def p1_fp(ci):
                """Pool-side front: the full subtract (halves for earlier exp)."""
                x, m = front_state[ci]
                xv = x[:].rearrange("p (g b) -> p g b", b=BASE)
                d = dp.tile([P, CW], F32, tag="d")
                dv = d[:].rearrange("p (g b) -> p g b", b=BASE)
                mb = m[:].unsqueeze(2).broadcast_to((P, GRP, BASE))
                nc.gpsimd.tensor_tensor(dv[:, :SG], xv[:, :SG], mb[:, :SG], ALU.subtract)
                nc.gpsimd.tensor_tensor(dv[:, SG:], xv[:, SG:], mb[:, SG:], ALU.subtract)
                front_state[ci] = (x, m, d)

            def p1_e(ci):
                """exp halves; the DVE-subtract half (upper) is ready first."""
                x, m, d = front_state[ci]
                e = xp.tile([P, CW], F32, tag="x")
                HW = CW // 2
                nc.scalar.activation(e[:, :HW], d[:, :HW], ACTF.Exp, scale=INV_TEMP)
                nc.scalar.activation(e[:, HW:], d[:, HW:], ACTF.Exp, scale=INV_TEMP)
                front_state[ci] = e

            def p1_bp(ci):
                """weight-mult on DVE: keeps the num-reduce dependency engine-local."""
                e = front_state[ci]
                w = dp.tile([P, CW], F32, tag="d")
                wv = w[:].rearrange("p (r q) -> p r q", q=DS * BASE)
                ev8 = e[:].rearrange("p (r q) -> p r q", q=DS * BASE)
                wb = wtile[:].unsqueeze(1).broadcast_to((P, CH * NI, DS * BASE))
                nc.vector.tensor_tensor(wv[:, :HR], ev8[:, :HR], wb[:, :HR], ALU.mult)
                nc.vector.tensor_tensor(wv[:, HR:], ev8[:, HR:], wb[:, HR:], ALU.mult)
                front_state[ci] = (e, w)

            def p1_bd(ci):
                """DVE back: den/num reduces (early halves first) + reciprocal."""
                e, w = front_state.pop(ci)
                den = denall[:, ci * GRP:(ci + 1) * GRP]
                ev = e[:].rearrange("p (g b) -> p g b", b=BASE)
                HG = GRP // 2
                nc.vector.tensor_reduce(den[:, :HG], ev[:, :HG], AX.X, ALU.add)
                nc.vector.tensor_reduce(den[:, HG:], ev[:, HG:], AX.X, ALU.add)
                num = numall[:, ci * GRP:(ci + 1) * GRP]
                w3 = w[:].rearrange("p (g b) -> p g b", b=BASE)
                nc.vector.tensor_reduce(num[:, :HG], w3[:, :HG], AX.X, ALU.add)
                nc.vector.tensor_reduce(num[:, HG:], w3[:, HG:], AX.X, ALU.add)
                rcp = sp.tile([P, GRP], F32, tag="rcp")
                scr = sp.tile([P, GRP], F32, tag="scr")
                nc.vector.reciprocal_approx_accurate(rcp[:], den[:], scr[:])
                return rcp

            def p1_tail(ci, rcp):
                """expected value + pow-weighted sum (Pool) + clip (DVE)."""
                num = numall[:, ci * GRP:(ci + 1) * GRP]
                ex = sp.tile([P, GRP], F32, tag="ex")
                nc.gpsimd.tensor_tensor(ex[:], num, rcp[:], ALU.mult)
                ex3 = ex[:].rearrange("p (r d) -> p r d", d=DS)
                v4 = sp.tile([P, CH * NI * 4], F32, tag="v4")
                v43 = v4[:].rearrange("p (r d) -> p r d", d=4)
                nc.gpsimd.tensor_tensor(v43, ex3[:, :, 0:4], ex3[:, :, 4:8], ALU.add)
                v2 = sp.tile([P, CH * NI * 2], F32, tag="v2")
                v23 = v2[:].rearrange("p (r d) -> p r d", d=2)
                nc.gpsimd.tensor_tensor(v23, v43[:, :, 0:2], v43[:, :, 2:4], ALU.add)
                vm = sp.tile([P, CH * NI], F32, tag="vm")
                nc.gpsimd.tensor_tensor(vm[:], v23[:, :, 0], v23[:, :, 1], ALU.add)
                nc.vector.tensor_scalar(
                    vmag[:, ci * CH * NI:(ci + 1) * CH * NI], vm[:],
                    MAG_MIN, MAG_MAX, ALU.max, ALU.min,
                )

            p1_fd(0)
            p1_fp(0)
            p1_e(0)
            p1_fd(1)
            # phase-2 inputs after the first chunk DMAs so they don't block them
            nc.sync.dma_start(otile[:], od)
            nc.sync.dma_start(wsign[:], vsg)
            nc.sync.dma_start(gtile[:], gd)
            nc.scalar.activation(absO2[:], otile[:], ACTF.Abs, scale=2.0)
            nc.vector.tensor_scalar(onemg[:], gtile[:], -1.0, 1.0, ALU.mult, ALU.add)
            for ci in range(NCHUNK):
                p1_bp(ci)
                rcp = p1_bd(ci)
                if ci + 1 < NCHUNK:
                    p1_fp(ci + 1)
                    p1_e(ci + 1)
                if ci + 2 < NCHUNK:
                    p1_fd(ci + 2)
                p1_tail(ci, rcp)

            # ---- phase 2: DAG recurrence ----
            wm3 = wmag[:].rearrange("p (t n) -> p t n", n=TN)
            nc.vector.tensor_copy(
                wm3[:, :, 0:NI], vmag[:].rearrange("p (t n) -> p t n", n=NI)
            )
            nc.vector.tensor_tensor(signed[:], wsign[:], wmag[:], ALU.mult)
            _emit_ln(nc, stp, logm[:], wmag[:], TP * TN, "lni")
            dc3 = diffc[:].rearrange("p (t n) -> p t n", n=TN)
            nc.vector.tensor_tensor(diffc[:], signed[:], logm[:], ALU.subtract)
            ws3 = wsign[:].rearrange("p (t n) -> p t n", n=TN)
            sg3 = signed[:].rearrange("p (t n) -> p t n", n=TN)
            lg3 = logm[:].rearrange("p (t n) -> p t n", n=TN)
            o4 = otile[:].rearrange("p (t s n) -> p t s n", s=DD, n=TN)
            a4 = absO2[:].rearrange("p (t s n) -> p t s n", s=DD, n=TN)


            def gs(ap, s, n_bcast=None):
                v = ap[:, s::DD]
                if n_bcast is None:
                    return v
                return v.unsqueeze(2).broadcast_to((P, TP, n_bcast))

            def p2_front(s):
                """Heavy dots over nodes [0, K): everything but the node written
                by step s-1.  Depends only on >=2-step-old state, so it runs one
                step ahead of its consumer."""
                K = NI if s == 0 else NI - 1 + s
                t1 = stp.tile([P, TP * TN], F32, tag="t1f")
                t13 = t1[:].rearrange("p (t n) -> p t n", n=TN)
                nc.gpsimd.tensor_tensor(
                    t13[:, :, :K], dc3[:, :, :K], gs(gtile[:], s, K), ALU.mult
                )
                mx = stp.tile([P, TP * TN], F32, tag="mxf")
                mx3 = mx[:].rearrange("p (t n) -> p t n", n=TN)
                nc.vector.tensor_tensor(
                    mx3[:, :, :K], lg3[:, :, :K], t13[:, :, :K], ALU.add
                )
                rt = stp.tile([P, TP * TN], F32, tag="rtf")
                rt3 = rt[:].rearrange("p (t n) -> p t n", n=TN)
                nc.vector.tensor_tensor(
                    rt3[:, :, :K], mx3[:, :, :K], o4[:, :, s, :K], ALU.mult
                )
                rold = st2.tile([P, TP], F32, tag="rold")
                nc.vector.tensor_reduce(rold[:], rt3[:, :, :K], AX.X, ALU.add)

                sw = stp.tile([P, TP * TN], F32, tag="swf")
                sw3 = sw[:].rearrange("p (t n) -> p t n", n=TN)
                nc.gpsimd.tensor_tensor(
                    sw3[:, :, :K], ws3[:, :, :K], a4[:, :, s, :K], ALU.mult
                )
                swp = stp.tile([P, TP * TN], F32, tag="swpf")
                swp3 = swp[:].rearrange("p (t n) -> p t n", n=TN)
                nc.scalar.activation(swp3[:, :, :K], sw3[:, :, :K], ACTF.Copy, bias=1.0)
                # product over the K nodes: pairwise multiply tree on Pool
                ta = st2.tile([P, TP * TN], F32, tag="ta")
                tb = st2.tile([P, TP * TN], F32, tag="tb")
                pbufs = [
                    ta[:].rearrange("p (t n) -> p t n", n=TN),
                    tb[:].rearrange("p (t n) -> p t n", n=TN),
                ]
                src3, width, pi = swp3, K, 0
                while width > 1:
                    half, odd = width // 2, width % 2
                    dst3 = pbufs[pi]
                    nc.gpsimd.tensor_tensor(
                        dst3[:, :, :half], src3[:, :, :half],
                        src3[:, :, half:2 * half], ALU.mult,
                    )
                    if odd:
                        nc.vector.tensor_copy(dst3[:, :, half], src3[:, :, 2 * half])
                    src3, width, pi = dst3, half + odd, 1 - pi
                return rold, src3[:, :, 0]

            res = sp.tile([P, TP], F32, tag="res")
            fr = p2_front(0)

            for s in range(DD):
                last = s == DD - 1
                rold, prodold = fr

                rp = st2.tile([P, 2 * TP], F32, tag="rp")  # [R | prod]
                if s == 0:
                    nc.vector.tensor_copy(rp[:, 0:TP], rold[:])
                    nc.vector.tensor_copy(rp[:, TP:2 * TP], prodold)
                else:
                    # fold in the node written by step s-1 (index NI-1+s)
                    nd = NI - 1 + s
                    q1 = stp.tile([P, TP], F32, tag="q1")
                    nc.vector.tensor_tensor(
                        q1[:], lg3[:, :, nd], gs(onemg[:], s), ALU.mult
                    )
                    q2 = stp.tile([P, TP], F32, tag="q2")
                    nc.vector.tensor_tensor(
                        q2[:], sg3[:, :, nd], gs(gtile[:], s), ALU.mult
                    )
                    mixn = stp.tile([P, TP], F32, tag="mixn")
                    nc.vector.tensor_tensor(mixn[:], q1[:], q2[:], ALU.add)
                    rn = stp.tile([P, TP], F32, tag="rn")
                    nc.vector.tensor_tensor(rn[:], mixn[:], o4[:, :, s, nd], ALU.mult)
                    nc.vector.tensor_tensor(rp[:, 0:TP], rold[:], rn[:], ALU.add)
                    swn = stp.tile([P, TP], F32, tag="swn")
                    nc.vector.tensor_tensor(swn[:], ws3[:, :, nd], a4[:, :, s, nd], ALU.mult)
                    nc.vector.tensor_scalar(swn[:], swn[:], 1.0, None, ALU.add)
                    nc.vector.tensor_tensor(rp[:, TP:2 * TP], prodold, swn[:], ALU.mult)

                # prefetch next step's heavy dots while this step's tail runs
                if not last:
                    fr = p2_front(s + 1)

                # tanh(y/SIGN_TEMP) = 1 - 2/(1+exp(2e4*y)) on [R | prod] at once
                yc = stp.tile([P, 2 * TP], F32, tag="yc")
                nc.vector.tensor_scalar(yc[:], rp[:], -0.005, 0.005, ALU.max, ALU.min)
                tE = stp.tile([P, 2 * TP], F32, tag="tE")
                nc.scalar.activation(tE[:], yc[:], ACTF.Exp, scale=2.0 * INV_SIGN_TEMP)
                u = stp.tile([P, 2 * TP], F32, tag="u")
                nc.vector.tensor_scalar(u[:], tE[:], 1.0, 1e30, ALU.add, ALU.min)
                rc2 = stp.tile([P, 2 * TP], F32, tag="rc2")
                sc2 = stp.tile([P, 2 * TP], F32, tag="sc2")
                nc.vector.reciprocal_approx_accurate(rc2[:], u[:], sc2[:])
                th = stp.tile([P, 2 * TP], F32, tag="th")
                nc.vector.tensor_scalar(th[:], rc2[:], -2.0, 1.0, ALU.mult, ALU.add)
                lin_sign, log_sign = th[:, 0:TP], th[:, TP:2 * TP]

                # s_new = G*lin + (1-G)*log  (clip dropped: convex combo of
                # values in [-1,1] stays within 1ulp of the range)
                sa = stp.tile([P, TP], F32, tag="sa")
                nc.vector.tensor_tensor(sa[:], lin_sign, gs(gtile[:], s), ALU.mult)
                sb = stp.tile([P, TP], F32, tag="sb")
                nc.vector.tensor_tensor(sb[:], log_sign, gs(onemg[:], s), ALU.mult)
                snew = stp.tile([P, TP], F32, tag="snew")
                nc.vector.tensor_tensor(snew[:], sa[:], sb[:], ALU.add)
                nc.vector.tensor_scalar(snew[:], snew[:], -1.0, 1.0, ALU.max, ALU.min)

                # m_new = G*min(|R|,MAX) + (1-G)*exp(clip(R,-100,EXP_CLAMP)), clipped.
                # The EXP_CLAMP upper bound replaces the reference's inf path: any
                # clamped value yields (1-G)*e^88.72 >= 2e31, which the final clip
                # maps to 1e28 exactly as inf would.
                R = rp[:, 0:TP]
                absR = stp.tile([P, TP], F32, tag="absR")
                nc.scalar.activation(absR[:], R, ACTF.Abs)
                nc.vector.tensor_scalar(absR[:], absR[:], MAG_MAX, None, ALU.min)
                rc = stp.tile([P, TP], F32, tag="rc")
                nc.vector.tensor_scalar(rc[:], R, -LOG_LIM, EXP_CLAMP, ALU.max, ALU.min)
                logres = stp.tile([P, TP], F32, tag="logres")
                nc.scalar.activation(logres[:], rc[:], ACTF.Exp)
                ma = stp.tile([P, TP], F32, tag="ma")
                nc.vector.tensor_tensor(ma[:], absR[:], gs(gtile[:], s), ALU.mult)
                mb2 = stp.tile([P, TP], F32, tag="mb2")
                nc.vector.tensor_tensor(mb2[:], logres[:], gs(onemg[:], s), ALU.mult)
                mnew = stp.tile([P, TP], F32, tag="mnew")
                nc.vector.tensor_tensor(mnew[:], ma[:], mb2[:], ALU.add)
                nc.vector.tensor_scalar(mnew[:], mnew[:], MAG_MIN, MAG_MAX, ALU.max, ALU.min)

                if last:
                    # output = wsign[16] * wmag[16]; skip the state writes
                    nc.vector.tensor_tensor(res[:], mnew[:], snew[:], ALU.mult)
                else:
                    ni = NI + s
                    nc.vector.tensor_copy(wm3[:, :, ni], mnew[:])
                    nc.vector.tensor_copy(ws3[:, :, ni], snew[:])
                    nc.gpsimd.tensor_tensor(sg3[:, :, ni], mnew[:], snew[:], ALU.mult)
                    _emit_ln(nc, stp, lg3[:, :, ni], mnew[:], TP, "lns")
                    nc.vector.tensor_tensor(
                        dc3[:, :, ni], sg3[:, :, ni], lg3[:, :, ni], ALU.subtract
                    )

            nc.sync.dma_start(out, res[:])

    nc.compile()
    return nc


_NC_CACHE = None


def _get_nc():
    global _NC_CACHE
    if _NC_CACHE is None:
        _NC_CACHE = build_program()
    return _NC_CACHE


def make_in_maps(digit_logits, V_sign, O, G):
    dlf = np.ascontiguousarray(digit_logits, dtype=np.float32).reshape(B * T, PTOK)
    vsf = np.ascontiguousarray(V_sign, dtype=np.float32).reshape(B * T, TN)
    of = np.ascontiguousarray(O, dtype=np.float32).reshape(B * T, DD * TN)
    gf = np.ascontiguousarray(G, dtype=np.float32).reshape(B * T, DD)
    pat = np.zeros(DS * BASE, dtype=np.float32)
    for dd in range(DS):
        for i in range(BASE):
            pat[dd * BASE + i] = i * POWERS[dd]
    wpat = np.tile(pat[None, :], (P, 1))
    in_maps = []
    for c in range(NCORES):
        s0, s1 = c * TOK_PER_CORE, (c + 1) * TOK_PER_CORE
        in_maps.append({
            "dl": np.ascontiguousarray(dlf[s0:s1].reshape(P, TP * PTOK)),
            "vsg": np.ascontiguousarray(vsf[s0:s1].reshape(P, TP * TN)),
            "od": np.ascontiguousarray(of[s0:s1].reshape(P, TP * DD * TN)),
            "gd": np.ascontiguousarray(gf[s0:s1].reshape(P, TP * DD)),
            "wpat": wpat,
        })
    return in_maps


def kernel(digit_logits, V_sign, O, G, _trace=False, _return_results=False):
    nc = _get_nc()
    in_maps = make_in_maps(digit_logits, V_sign, O, G)
    res = run_bass_kernel_spmd(nc, in_maps, list(range(NCORES)), trace=_trace)
    outs = [np.asarray(res.results[c]["out"]).reshape(TOK_PER_CORE) for c in range(NCORES)]
    full = np.concatenate(outs).reshape(B, T)
    if _return_results:
        return full, res
    return full


# revision 45
# speedup vs baseline: 1.1657x; 1.0610x over previous
"""Trainium2 Bass kernel for nn_DAGExecutor (digit-softmax + 8-step DAG recurrence).

Fully data-parallel: 32768 (B*T) tokens sharded as 4096 tokens per core across
8 cores; no cross-core communication.  Per core tokens live as [128 partitions
x 32 tokens].

Engine split: ACT does exp/ln/abs; Pool (gpsimd) takes ~90% of the two big
phase-1 elementwise passes plus assorted multiplies; DVE owns the grouped
reduces and the small-op chains.  Phase 1 is software-pipelined (front/back
skew) because engine queues execute in program order.  Phase 2 pipelines each
step's heavy node-dots ("front", which only need state that is two steps old)
one step ahead, leaving a short per-token fixup chain on the critical path.
"""

import sys

for _p in ("/opt/trn_rl_repo",):
    if _p not in sys.path:
        sys.path.insert(0, _p)

import numpy as np

import concourse.bass as bass
import concourse.bacc as bacc
import concourse.mybir as mybir
import concourse.tile as tile
from concourse.bass_utils import run_bass_kernel_spmd

F32 = mybir.dt.float32
I32 = mybir.dt.int32
AX = mybir.AxisListType
ALU = mybir.AluOpType
ACTF = mybir.ActivationFunctionType

# ---- problem constants (hardcoded; must match reference setup) ----
B, T = 8, 4096
NI = 9          # initial nodes
DD = 8          # dag depth
TN = 17         # total nodes
DS = 8          # digit slots
BASE = 10
PTOK = 720      # values per token in digit_logits = NI*DS*BASE
NCORES = 8
TOK_PER_CORE = (B * T) // NCORES        # 4096
P = 128                                  # partitions
TP = TOK_PER_CORE // P                   # 32 tokens per partition
MAG_MIN, MAG_MAX = 1e-12, 1e28
LOG_LIM = 100.0
INV_TEMP = 100.0                         # 1/TEMP
INV_SIGN_TEMP = 1e4                      # 1/SIGN_TEMP
# f32 exp overflow boundary: clamping exp's input here yields a value big
# enough that the downstream [1e-12,1e28] clip matches the reference's inf path
EXP_CLAMP = 88.7228355
POWERS = np.asarray([float(BASE) ** (4 - 1 - d) for d in range(DS)], dtype=np.float32)
LN2 = float(np.log(np.float32(2.0)))

CH = 4                                   # tokens-per-partition per phase-1 chunk
NCHUNK = TP // CH                        # 8 chunks
GRP = CH * NI * DS                       # softmax groups per chunk (288)
CW = CH * PTOK                           # chunk width (2880)
SG = GRP // 2                            # Pool share of the subtract pass
HR = (CH * NI) // 2                      # half of the token*node axis


def _emit_ln(nc, pool, dst_ap, src_ap, width, tag):
    """dst = ln(src) for src in [1e-12, +inf] via exponent/mantissa split.

    ACT Ln only covers |x| <= 2^64 and direct rescaling costs accuracy near
    ln(x)=0; the split keeps ~2ulp everywhere."""
    xb = src_ap.bitcast(I32)
    eint = pool.tile([P, width], I32, tag=tag + "_e")
    nc.vector.tensor_scalar(eint[:], xb, 23, None, ALU.logical_shift_right)
    le = pool.tile([P, width], F32, tag=tag + "_le")
    nc.vector.tensor_scalar(le[:], eint[:], 127.0, LN2, ALU.subtract, ALU.mult)
    mbits = pool.tile([P, width], I32, tag=tag + "_mb")
    nc.vector.tensor_scalar(
        mbits[:], xb, 0x007FFFFF, 0x3F800000, ALU.bitwise_and, ALU.bitwise_or
    )
    lnm = pool.tile([P, width], F32, tag=tag + "_lm")
    nc.scalar.activation(lnm[:], mbits[:].bitcast(F32), ACTF.Ln)
    nc.vector.tensor_tensor(dst_ap, lnm[:], le[:], ALU.add)


def _patch_act_tables():
    """Force all activations onto the natural_log_exp_and_others table set.

    The table-load pass greedily alternates exp_and_others / natural_log,
    inserting ~18 ACT table loads (~2.7us each on HW).  Emptying every other
    set (indices preserved) makes the combined set the only candidate."""
    import concourse.hw_specs as hw_specs
    orig = hw_specs.get_activation_tables

    def patched(arch):
        tabs = orig(arch)
        keep = "natural_log_exp_and_others"
        if keep not in tabs:
            return tabs
        return {k: (v if k == keep else set()) for k, v in tabs.items()}

    patched.__wrapped__ = orig
    bacc.get_activation_tables = patched


def build_program():
    _patch_act_tables()
    nc = bacc.Bacc("TRN2", target_bir_lowering=False, debug=False)

    dl = nc.dram_tensor("dl", [P, TP * PTOK], F32, kind="ExternalInput").ap()
    vsg = nc.dram_tensor("vsg", [P, TP * TN], F32, kind="ExternalInput").ap()
    od = nc.dram_tensor("od", [P, TP * DD * TN], F32, kind="ExternalInput").ap()
    gd = nc.dram_tensor("gd", [P, TP * DD], F32, kind="ExternalInput").ap()
    wpat = nc.dram_tensor("wpat", [P, DS * BASE], F32, kind="ExternalInput").ap()
    out = nc.dram_tensor("out", [P, TP], F32, kind="ExternalOutput").ap()

    with tile.TileContext(nc) as tc:
        with (
            tc.tile_pool(name="persist", bufs=1) as pp,
            tc.tile_pool(name="xin", bufs=4) as xp,
            tc.tile_pool(name="dbuf", bufs=3) as dp,
            tc.tile_pool(name="small", bufs=3) as sp,
            tc.tile_pool(name="steps", bufs=1) as stp,
            tc.tile_pool(name="steps2", bufs=2) as st2,
        ):
            # ---- persistent tiles ----
            vmag = pp.tile([P, TP * NI], F32, tag="vmag")
            otile = pp.tile([P, TP * DD * TN], F32, tag="otile")
            wsign = pp.tile([P, TP * TN], F32, tag="wsign")
            gtile = pp.tile([P, TP * DD], F32, tag="gtile")
            wtile = pp.tile([P, DS * BASE], F32, tag="wtile")
            absO2 = pp.tile([P, TP * DD * TN], F32, tag="absO2")
            onemg = pp.tile([P, TP * DD], F32, tag="onemg")
            wmag = pp.tile([P, TP * TN], F32, tag="wmag")
            signed = pp.tile([P, TP * TN], F32, tag="signed")
            logm = pp.tile([P, TP * TN], F32, tag="logm")
            diffc = pp.tile([P, TP * TN], F32, tag="diffc")
            denall = pp.tile([P, NCHUNK * GRP], F32, tag="denall")
            numall = pp.tile([P, NCHUNK * GRP], F32, tag="numall")

            nc.sync.dma_start(wtile[:], wpat)
            nc.gpsimd.memset(wmag[:], MAG_MIN)

            # ---- phase 1: digit softmax expected value -> vmag ----
            front_state = {}

            def p1_fd(ci):
                """DVE-side front: dma, group max, DVE share of subtract."""
                x = xp.tile([P, CW], F32, tag="x")
                nc.sync.dma_start(x[:], dl[:, ci * CW:(ci + 1) * CW])
                xv = x[:].rearrange("p (g b) -> p g b", b=BASE)
                m = sp.tile([P, GRP], F32, tag="m")
                nc.vector.tensor_reduce(m[:, :SG], xv[:, :SG], AX.X, ALU.max)
                nc.vector.tensor_reduce(m[:, SG:], xv[:, SG:], AX.X, ALU.max)
                front_state[ci] = (x, m)

            def p1_fp(ci):
                """Pool-side front: the full subtract (halves for earlier exp)."""
                x, m = front_state[ci]
                xv = x[:].rearrange("p (g b) -> p g b", b=BASE)
                d = dp.tile([P, CW], F32, tag="d")
                dv = d[:].rearrange("p (g b) -> p g b", b=BASE)
                mb = m[:].unsqueeze(2).broadcast_to((P, GRP, BASE))
                nc.gpsimd.tensor_tensor(dv[:, :SG], xv[:, :SG], mb[:, :SG], ALU.subtract)
                nc.gpsimd.tensor_tensor(dv[:, SG:], xv[:, SG:], mb[:, SG:], ALU.subtract)
                front_state[ci] = (x, m, d)

            def p1_e(ci):
                """exp halves; the DVE-subtract half (upper) is ready first."""
                x, m, d = front_state[ci]
                e = xp.tile([P, CW], F32, tag="x")
                HW = CW // 2
                nc.scalar.activation(e[:, :HW], d[:, :HW], ACTF.Exp, scale=INV_TEMP)
                nc.scalar.activation(e[:, HW:], d[:, HW:], ACTF.Exp, scale=INV_TEMP)
                front_state[ci] = e

            def p1_bp(ci):
                """weight-mult on DVE: keeps the num-reduce dependency engine-local."""
                e = front_state[ci]
                w = dp.tile([P, CW], F32, tag="d")
                wv = w[:].rearrange("p (r q) -> p r q", q=DS * BASE)
                ev8 = e[:].rearrange("p (r q) -> p r q", q=DS * BASE)
                wb = wtile[:].unsqueeze(1).broadcast_to((P, CH * NI, DS * BASE))
                nc.vector.tensor_tensor(wv[:, :HR], ev8[:, :HR], wb[:, :HR], ALU.mult)
                nc.vector.tensor_tensor(wv[:, HR:], ev8[:, HR:], wb[:, HR:], ALU.mult)
                front_state[ci] = (e, w)

            def p1_bd(ci):
                """DVE back: den/num reduces (early halves first) + reciprocal."""
                e, w = front_state.pop(ci)
                den = denall[:, ci * GRP:(ci + 1) * GRP]
                ev = e[:].rearrange("p (g b) -> p g b", b=BASE)
                HG = GRP // 2
                nc.vector.tensor_reduce(den[:, :HG], ev[:, :HG], AX.X, ALU.add)
                nc.vector.tensor_reduce(den[:, HG:], ev[:, HG:], AX.X, ALU.add)
                num = numall[:, ci * GRP:(ci + 1) * GRP]
                w3 = w[:].rearrange("p (g b) -> p g b", b=BASE)
                nc.vector.tensor_reduce(num[:, :HG], w3[:, :HG], AX.X, ALU.add)
                nc.vector.tensor_reduce(num[:, HG:], w3[:, HG:], AX.X, ALU.add)
                rcp = sp.tile([P, GRP], F32, tag="rcp")
                scr = sp.tile([P, GRP], F32, tag="scr")
                nc.vector.reciprocal_approx_accurate(rcp[:], den[:], scr[:])
                return rcp

            def p1_tail(ci, rcp):
                """expected value + pow-weighted sum (Pool) + clip (DVE)."""
                num = numall[:, ci * GRP:(ci + 1) * GRP]
                ex = sp.tile([P, GRP], F32, tag="ex")
                nc.gpsimd.tensor_tensor(ex[:], num, rcp[:], ALU.mult)
                ex3 = ex[:].rearrange("p (r d) -> p r d", d=DS)
                v4 = sp.tile([P, CH * NI * 4], F32, tag="v4")
                v43 = v4[:].rearrange("p (r d) -> p r d", d=4)
                nc.gpsimd.tensor_tensor(v43, ex3[:, :, 0:4], ex3[:, :, 4:8], ALU.add)
                v2 = sp.tile([P, CH * NI * 2], F32, tag="v2")
                v23 = v2[:].rearrange("p (r d) -> p r d", d=2)
                nc.gpsimd.tensor_tensor(v23, v43[:, :, 0:2], v43[:, :, 2:4], ALU.add)
                vm = sp.tile([P, CH * NI], F32, tag="vm")
                nc.gpsimd.tensor_tensor(vm[:], v23[:, :, 0], v23[:, :, 1], ALU.add)
                nc.vector.tensor_scalar(
                    vmag[:, ci * CH * NI:(ci + 1) * CH * NI], vm[:],
                    MAG_MIN, MAG_MAX, ALU.max, ALU.min,
                )

            p1_fd(0)
            p1_fp(0)
            p1_e(0)
            p1_fd(1)
            # phase-2 inputs after the first chunk DMAs so they don't block them
            nc.sync.dma_start(otile[:], od)
            nc.sync.dma_start(wsign[:], vsg)
            nc.sync.dma_start(gtile[:], gd)
            nc.scalar.activation(absO2[:], otile[:], ACTF.Abs, scale=2.0)
            nc.vector.tensor_scalar(onemg[:], gtile[:], -1.0, 1.0, ALU.mult, ALU.add)
            for ci in range(NCHUNK):
                p1_bp(ci)
                rcp = p1_bd(ci)
                if ci + 1 < NCHUNK:
                    p1_fp(ci + 1)
                    p1_e(ci + 1)
                if ci + 2 < NCHUNK:
                    p1_fd(ci + 2)
                p1_tail(ci, rcp)

            # ---- phase 2: DAG recurrence ----
            wm3 = wmag[:].rearrange("p (t n) -> p t n", n=TN)
            nc.vector.tensor_copy(
                wm3[:, :, 0:NI], vmag[:].rearrange("p (t n) -> p t n", n=NI)
            )
            nc.vector.tensor_tensor(signed[:], wsign[:], wmag[:], ALU.mult)
            _emit_ln(nc, stp, logm[:], wmag[:], TP * TN, "lni")
            dc3 = diffc[:].rearrange("p (t n) -> p t n", n=TN)
            nc.vector.tensor_tensor(diffc[:], signed[:], logm[:], ALU.subtract)
            ws3 = wsign[:].rearrange("p (t n) -> p t n", n=TN)
            sg3 = signed[:].rearrange("p (t n) -> p t n", n=TN)
            lg3 = logm[:].rearrange("p (t n) -> p t n", n=TN)
            o4 = otile[:].rearrange("p (t s n) -> p t s n", s=DD, n=TN)
            a4 = absO2[:].rearrange("p (t s n) -> p t s n", s=DD, n=TN)


            def gs(ap, s, n_bcast=None):
                v = ap[:, s::DD]
                if n_bcast is None:
                    return v
                return v.unsqueeze(2).broadcast_to((P, TP, n_bcast))

            def p2_front(s):
                """Heavy dots over nodes [0, K): everything but the node written
                by step s-1.  Depends only on >=2-step-old state, so it runs one
                step ahead of its consumer."""
                K = NI if s == 0 else NI - 1 + s
                t1 = stp.tile([P, TP * TN], F32, tag="t1f")
                t13 = t1[:].rearrange("p (t n) -> p t n", n=TN)
                nc.gpsimd.tensor_tensor(
                    t13[:, :, :K], dc3[:, :, :K], gs(gtile[:], s, K), ALU.mult
                )
                mx = stp.tile([P, TP * TN], F32, tag="mxf")
                mx3 = mx[:].rearrange("p (t n) -> p t n", n=TN)
                nc.vector.tensor_tensor(
                    mx3[:, :, :K], lg3[:, :, :K], t13[:, :, :K], ALU.add
                )
                rt = stp.tile([P, TP * TN], F32, tag="rtf")
                rt3 = rt[:].rearrange("p (t n) -> p t n", n=TN)
                nc.vector.tensor_tensor(
                    rt3[:, :, :K], mx3[:, :, :K], o4[:, :, s, :K], ALU.mult
                )
                rold = st2.tile([P, TP], F32, tag="rold")
                nc.vector.tensor_reduce(rold[:], rt3[:, :, :K], AX.X, ALU.add)

                sw = stp.tile([P, TP * TN], F32, tag="swf")
                sw3 = sw[:].rearrange("p (t n) -> p t n", n=TN)
                nc.gpsimd.tensor_tensor(
                    sw3[:, :, :K], ws3[:, :, :K], a4[:, :, s, :K], ALU.mult
                )
                swp = stp.tile([P, TP * TN], F32, tag="swpf")
                swp3 = swp[:].rearrange("p (t n) -> p t n", n=TN)
                nc.scalar.activation(swp3[:, :, :K], sw3[:, :, :K], ACTF.Copy, bias=1.0)
                # product over the K nodes: pairwise multiply tree on Pool
                ta = st2.tile([P, TP * TN], F32, tag="ta")
                tb = st2.tile([P, TP * TN], F32, tag="tb")
                pbufs = [
                    ta[:].rearrange("p (t n) -> p t n", n=TN),
                    tb[:].rearrange("p (t n) -> p t n", n=TN),
                ]
                src3, width, pi = swp3, K, 0
                while width > 1:
                    half, odd = width // 2, width % 2
                    dst3 = pbufs[pi]
                    nc.gpsimd.tensor_tensor(
                        dst3[:, :, :half], src3[:, :, :half],
                        src3[:, :, half:2 * half], ALU.mult,
                    )
                    if odd:
                        nc.vector.tensor_copy(dst3[:, :, half], src3[:, :, 2 * half])
                    src3, width, pi = dst3, half + odd, 1 - pi
                return rold, src3[:, :, 0]

            res = sp.tile([P, TP], F32, tag="res")
            fr = p2_front(0)

            for s in range(DD):
                last = s == DD - 1
                rold, prodold = fr

                rp = st2.tile([P, 2 * TP], F32, tag="rp")  # [R | prod]
                if s == 0:
                    nc.vector.tensor_copy(rp[:, 0:TP], rold[:])
                    nc.vector.tensor_copy(rp[:, TP:2 * TP], prodold)
                else:
                    # fold in the node written by step s-1 (index NI-1+s)
                    nd = NI - 1 + s
                    q1 = stp.tile([P, TP], F32, tag="q1")
                    nc.vector.tensor_tensor(
                        q1[:], lg3[:, :, nd], gs(onemg[:], s), ALU.mult
                    )
                    q2 = stp.tile([P, TP], F32, tag="q2")
                    nc.vector.tensor_tensor(
                        q2[:], sg3[:, :, nd], gs(gtile[:], s), ALU.mult
                    )
                    mixn = stp.tile([P, TP], F32, tag="mixn")
                    nc.vector.tensor_tensor(mixn[:], q1[:], q2[:], ALU.add)
                    rn = stp.tile([P, TP], F32, tag="rn")
                    nc.vector.tensor_tensor(rn[:], mixn[:], o4[:, :, s, nd], ALU.mult)
                    nc.vector.tensor_tensor(rp[:, 0:TP], rold[:], rn[:], ALU.add)
                    swn = stp.tile([P, TP], F32, tag="swn")
                    nc.vector.tensor_tensor(swn[:], ws3[:, :, nd], a4[:, :, s, nd], ALU.mult)
                    nc.vector.tensor_scalar(swn[:], swn[:], 1.0, None, ALU.add)
                    nc.vector.tensor_tensor(rp[:, TP:2 * TP], prodold, swn[:], ALU.mult)

                # prefetch next step's heavy dots while this step's tail runs
                if not last:
                    fr = p2_front(s + 1)

                # tanh(y/SIGN_TEMP) = 1 - 2/(1+exp(2e4*y)) on [R | prod] at once
                yc = stp.tile([P, 2 * TP], F32, tag="yc")
                nc.vector.tensor_scalar(yc[:], rp[:], -0.005, 0.005, ALU.max, ALU.min)
                tE = stp.tile([P, 2 * TP], F32, tag="tE")
                nc.scalar.activation(tE[:], yc[:], ACTF.Exp, scale=2.0 * INV_SIGN_TEMP)
                u = stp.tile([P, 2 * TP], F32, tag="u")
                nc.vector.tensor_scalar(u[:], tE[:], 1.0, 1e30, ALU.add, ALU.min)
                rc2 = stp.tile([P, 2 * TP], F32, tag="rc2")
                sc2 = stp.tile([P, 2 * TP], F32, tag="sc2")
                nc.vector.reciprocal_approx_accurate(rc2[:], u[:], sc2[:])
                th = stp.tile([P, 2 * TP], F32, tag="th")
                nc.vector.tensor_scalar(th[:], rc2[:], -2.0, 1.0, ALU.mult, ALU.add)
                lin_sign, log_sign = th[:, 0:TP], th[:, TP:2 * TP]

                # s_new = G*lin + (1-G)*log  (clip dropped: convex combo of
                # values in [-1,1] stays within 1ulp of the range)
                sa = stp.tile([P, TP], F32, tag="sa")
                nc.vector.tensor_tensor(sa[:], lin_sign, gs(gtile[:], s), ALU.mult)
                sb = stp.tile([P, TP], F32, tag="sb")
                nc.vector.tensor_tensor(sb[:], log_sign, gs(onemg[:], s), ALU.mult)
                snew = stp.tile([P, TP], F32, tag="snew")
                nc.vector.tensor_tensor(snew[:], sa[:], sb[:], ALU.add)
                nc.vector.tensor_scalar(snew[:], snew[:], -1.0, 1.0, ALU.max, ALU.min)

                # m_new = G*min(|R|,MAX) + (1-G)*exp(clip(R,-100,EXP_CLAMP)), clipped.
                # The EXP_CLAMP upper bound replaces the reference's inf path: any
                # clamped value yields (1-G)*e^88.72 >= 2e31, which the final clip
                # maps to 1e28 exactly as inf would.
                R = rp[:, 0:TP]
                absR = stp.tile([P, TP], F32, tag="absR")
                nc.scalar.activation(absR[:], R, ACTF.Abs)
                nc.vector.tensor_scalar(absR[:], absR[:], MAG_MAX, None, ALU.min)
                rc = stp.tile([P, TP], F32, tag="rc")
                nc.vector.tensor_scalar(rc[:], R, -LOG_LIM, EXP_CLAMP, ALU.max, ALU.min)
                logres = stp.tile([P, TP], F32, tag="logres")
                nc.scalar.activation(logres[:], rc[:], ACTF.Exp)
                ma = stp.tile([P, TP], F32, tag="ma")
                nc.vector.tensor_tensor(ma[:], absR[:], gs(gtile[:], s), ALU.mult)
                mb2 = stp.tile([P, TP], F32, tag="mb2")
                nc.vector.tensor_tensor(mb2[:], logres[:], gs(onemg[:], s), ALU.mult)
                mnew = stp.tile([P, TP], F32, tag="mnew")
                nc.vector.tensor_tensor(mnew[:], ma[:], mb2[:], ALU.add)
                nc.vector.tensor_scalar(mnew[:], mnew[:], MAG_MIN, MAG_MAX, ALU.max, ALU.min)

                if last:
                    # output = wsign[16] * wmag[16]; skip the state writes
                    nc.vector.tensor_tensor(res[:], mnew[:], snew[:], ALU.mult)
                else:
                    ni = NI + s
                    nc.vector.tensor_copy(wm3[:, :, ni], mnew[:])
                    nc.vector.tensor_copy(ws3[:, :, ni], snew[:])
                    nc.gpsimd.tensor_tensor(sg3[:, :, ni], mnew[:], snew[:], ALU.mult)
                    _emit_ln(nc, stp, lg3[:, :, ni], mnew[:], TP, "lns")
                    nc.vector.tensor_tensor(
                        dc3[:, :, ni], sg3[:, :, ni], lg3[:, :, ni], ALU.subtract
                    )

            nc.sync.dma_start(out, res[:])

    nc.compile()
    return nc


_NC_CACHE = None


def _get_nc():
    global _NC_CACHE
    if _NC_CACHE is None:
        _NC_CACHE = build_program()
    return _NC_CACHE


def make_in_maps(digit_logits, V_sign, O, G):
    dlf = np.ascontiguousarray(digit_logits, dtype=np.float32).reshape(B * T, PTOK)
    vsf = np.ascontiguousarray(V_sign, dtype=np.float32).reshape(B * T, TN)
    of = np.ascontiguousarray(O, dtype=np.float32).reshape(B * T, DD * TN)
    gf = np.ascontiguousarray(G, dtype=np.float32).reshape(B * T, DD)
    pat = np.zeros(DS * BASE, dtype=np.float32)
    for dd in range(DS):
        for i in range(BASE):
            pat[dd * BASE + i] = i * POWERS[dd]
    wpat = np.tile(pat[None, :], (P, 1))
    in_maps = []
    for c in range(NCORES):
        s0, s1 = c * TOK_PER_CORE, (c + 1) * TOK_PER_CORE
        in_maps.append({
            "dl": np.ascontiguousarray(dlf[s0:s1].reshape(P, TP * PTOK)),
            "vsg": np.ascontiguousarray(vsf[s0:s1].reshape(P, TP * TN)),
            "od": np.ascontiguousarray(of[s0:s1].reshape(P, TP * DD * TN)),
            "gd": np.ascontiguousarray(gf[s0:s1].reshape(P, TP * DD)),
            "wpat": wpat,
        })
    return in_maps


def kernel(digit_logits, V_sign, O, G, _trace=False, _return_results=False):
    nc = _get_nc()
    in_maps = make_in_maps(digit_logits, V_sign, O, G)
    res = run_bass_kernel_spmd(nc, in_maps, list(range(NCORES)), trace=_trace)
    outs = [np.asarray(res.results[c]["out"]).reshape(TOK_PER_CORE) for c in range(NCORES)]
    full = np.concatenate(outs).reshape(B, T)
    if _return_results:
        return full, res
    return full


# revision 46
# speedup vs baseline: 1.1772x; 1.0099x over previous
"""Trainium2 Bass kernel for nn_DAGExecutor (digit-softmax + 8-step DAG recurrence).

Fully data-parallel: 32768 (B*T) tokens sharded as 4096 tokens per core across
8 cores; no cross-core communication.  Per core tokens live as [128 partitions
x 32 tokens].

Engine split: ACT does exp/ln/abs; Pool (gpsimd) takes ~90% of the two big
phase-1 elementwise passes plus assorted multiplies; DVE owns the grouped
reduces and the small-op chains.  Phase 1 is software-pipelined (front/back
skew) because engine queues execute in program order.  Phase 2 pipelines each
step's heavy node-dots ("front", which only need state that is two steps old)
one step ahead, leaving a short per-token fixup chain on the critical path.
"""

import sys

for _p in ("/opt/trn_rl_repo",):
    if _p not in sys.path:
        sys.path.insert(0, _p)

import numpy as np

import concourse.bass as bass
import concourse.bacc as bacc
import concourse.mybir as mybir
import concourse.tile as tile
from concourse.bass_utils import run_bass_kernel_spmd

F32 = mybir.dt.float32
I32 = mybir.dt.int32
AX = mybir.AxisListType
ALU = mybir.AluOpType
ACTF = mybir.ActivationFunctionType

# ---- problem constants (hardcoded; must match reference setup) ----
B, T = 8, 4096
NI = 9          # initial nodes
DD = 8          # dag depth
TN = 17         # total nodes
DS = 8          # digit slots
BASE = 10
PTOK = 720      # values per token in digit_logits = NI*DS*BASE
NCORES = 8
TOK_PER_CORE = (B * T) // NCORES        # 4096
P = 128                                  # partitions
TP = TOK_PER_CORE // P                   # 32 tokens per partition
MAG_MIN, MAG_MAX = 1e-12, 1e28
LOG_LIM = 100.0
INV_TEMP = 100.0                         # 1/TEMP
INV_SIGN_TEMP = 1e4                      # 1/SIGN_TEMP
# f32 exp overflow boundary: clamping exp's input here yields a value big
# enough that the downstream [1e-12,1e28] clip matches the reference's inf path
EXP_CLAMP = 88.7228355
POWERS = np.asarray([float(BASE) ** (4 - 1 - d) for d in range(DS)], dtype=np.float32)
LN2 = float(np.log(np.float32(2.0)))

CH = 4                                   # tokens-per-partition per phase-1 chunk
NCHUNK = TP // CH                        # 8 chunks
GRP = CH * NI * DS                       # softmax groups per chunk (288)
CW = CH * PTOK                           # chunk width (2880)
SG = GRP // 2                            # Pool share of the subtract pass
HR = (CH * NI) // 2                      # half of the token*node axis


def _emit_ln(nc, pool, dst_ap, src_ap, width, tag):
    """dst = ln(src) for src in [1e-12, +inf] via exponent/mantissa split.

    ACT Ln only covers |x| <= 2^64 and direct rescaling costs accuracy near
    ln(x)=0; the split keeps ~2ulp everywhere."""
    xb = src_ap.bitcast(I32)
    eint = pool.tile([P, width], I32, tag=tag + "_e")
    nc.vector.tensor_scalar(eint[:], xb, 23, None, ALU.logical_shift_right)
    le = pool.tile([P, width], F32, tag=tag + "_le")
    nc.vector.tensor_scalar(le[:], eint[:], 127.0, LN2, ALU.subtract, ALU.mult)
    mbits = pool.tile([P, width], I32, tag=tag + "_mb")
    nc.vector.tensor_scalar(
        mbits[:], xb, 0x007FFFFF, 0x3F800000, ALU.bitwise_and, ALU.bitwise_or
    )
    lnm = pool.tile([P, width], F32, tag=tag + "_lm")
    nc.scalar.activation(lnm[:], mbits[:].bitcast(F32), ACTF.Ln)
    nc.vector.tensor_tensor(dst_ap, lnm[:], le[:], ALU.add)


def _patch_act_tables():
    """Force all activations onto the natural_log_exp_and_others table set.

    The table-load pass greedily alternates exp_and_others / natural_log,
    inserting ~18 ACT table loads (~2.7us each on HW).  Emptying every other
    set (indices preserved) makes the combined set the only candidate."""
    import concourse.hw_specs as hw_specs
    orig = hw_specs.get_activation_tables

    def patched(arch):
        tabs = orig(arch)
        keep = "natural_log_exp_and_others"
        if keep not in tabs:
            return tabs
        return {k: (v if k == keep else set()) for k, v in tabs.items()}

    patched.__wrapped__ = orig
    bacc.get_activation_tables = patched


def build_program():
    _patch_act_tables()
    nc = bacc.Bacc("TRN2", target_bir_lowering=False, debug=False)

    dl = nc.dram_tensor("dl", [P, TP * PTOK], F32, kind="ExternalInput").ap()
    vsg = nc.dram_tensor("vsg", [P, TP * TN], F32, kind="ExternalInput").ap()
    od = nc.dram_tensor("od", [P, TP * DD * TN], F32, kind="ExternalInput").ap()
    gd = nc.dram_tensor("gd", [P, TP * DD], F32, kind="ExternalInput").ap()
    wpat = nc.dram_tensor("wpat", [P, DS * BASE], F32, kind="ExternalInput").ap()
    out = nc.dram_tensor("out", [P, TP], F32, kind="ExternalOutput").ap()

    with tile.TileContext(nc) as tc:
        with (
            tc.tile_pool(name="persist", bufs=1) as pp,
            tc.tile_pool(name="xin", bufs=4) as xp,
            tc.tile_pool(name="dbuf", bufs=3) as dp,
            tc.tile_pool(name="small", bufs=3) as sp,
            tc.tile_pool(name="steps", bufs=1) as stp,
            tc.tile_pool(name="steps2", bufs=2) as st2,
        ):
            # ---- persistent tiles ----
            vmag = pp.tile([P, TP * NI], F32, tag="vmag")
            otile = pp.tile([P, TP * DD * TN], F32, tag="otile")
            wsign = pp.tile([P, TP * TN], F32, tag="wsign")
            gtile = pp.tile([P, TP * DD], F32, tag="gtile")
            wtile = pp.tile([P, DS * BASE], F32, tag="wtile")
            absO2 = pp.tile([P, TP * DD * TN], F32, tag="absO2")
            onemg = pp.tile([P, TP * DD], F32, tag="onemg")
            wmag = pp.tile([P, TP * TN], F32, tag="wmag")
            signed = pp.tile([P, TP * TN], F32, tag="signed")
            logm = pp.tile([P, TP * TN], F32, tag="logm")
            diffc = pp.tile([P, TP * TN], F32, tag="diffc")
            denall = pp.tile([P, NCHUNK * GRP], F32, tag="denall")
            numall = pp.tile([P, NCHUNK * GRP], F32, tag="numall")

            nc.sync.dma_start(wtile[:], wpat)
            nc.gpsimd.memset(wmag[:], MAG_MIN)

            # ---- phase 1: digit softmax expected value -> vmag ----
            front_state = {}

            def p1_fd(ci):
                """DVE-side front: dma, group max, DVE share of subtract."""
                x = xp.tile([P, CW], F32, tag="x")
                nc.sync.dma_start(x[:], dl[:, ci * CW:(ci + 1) * CW])
                xv = x[:].rearrange("p (g b) -> p g b", b=BASE)
                m = sp.tile([P, GRP], F32, tag="m")
                Q = GRP // 4
                for qi in range(4):
                    nc.vector.tensor_reduce(
                        m[:, qi * Q:(qi + 1) * Q], xv[:, qi * Q:(qi + 1) * Q],
                        AX.X, ALU.max,
                    )
                front_state[ci] = (x, m)

            def p1_fp(ci):
                """Pool-side front: the full subtract (halves for earlier exp)."""
                x, m = front_state[ci]
                xv = x[:].rearrange("p (g b) -> p g b", b=BASE)
                d = dp.tile([P, CW], F32, tag="d")
                dv = d[:].rearrange("p (g b) -> p g b", b=BASE)
                mb = m[:].unsqueeze(2).broadcast_to((P, GRP, BASE))
                Q = GRP // 4
                for qi in range(4):
                    sl = slice(qi * Q, (qi + 1) * Q)
                    nc.gpsimd.tensor_tensor(dv[:, sl], xv[:, sl], mb[:, sl], ALU.subtract)
                front_state[ci] = (x, m, d)

            def p1_e(ci):
                """exp halves; the DVE-subtract half (upper) is ready first."""
                x, m, d = front_state[ci]
                e = xp.tile([P, CW], F32, tag="x")
                HW = CW // 2
                nc.scalar.activation(e[:, :HW], d[:, :HW], ACTF.Exp, scale=INV_TEMP)
                nc.scalar.activation(e[:, HW:], d[:, HW:], ACTF.Exp, scale=INV_TEMP)
                front_state[ci] = e

            def p1_bp(ci):
                """weight-mult on DVE: keeps the num-reduce dependency engine-local."""
                e = front_state[ci]
                w = dp.tile([P, CW], F32, tag="d")
                wv = w[:].rearrange("p (r q) -> p r q", q=DS * BASE)
                ev8 = e[:].rearrange("p (r q) -> p r q", q=DS * BASE)
                wb = wtile[:].unsqueeze(1).broadcast_to((P, CH * NI, DS * BASE))
                nc.vector.tensor_tensor(wv[:, :HR], ev8[:, :HR], wb[:, :HR], ALU.mult)
                nc.vector.tensor_tensor(wv[:, HR:], ev8[:, HR:], wb[:, HR:], ALU.mult)
                front_state[ci] = (e, w)

            def p1_bd(ci):
                """DVE back: den/num reduces (early halves first) + reciprocal."""
                e, w = front_state.pop(ci)
                den = denall[:, ci * GRP:(ci + 1) * GRP]
                ev = e[:].rearrange("p (g b) -> p g b", b=BASE)
                HG = GRP // 2
                nc.vector.tensor_reduce(den[:, :HG], ev[:, :HG], AX.X, ALU.add)
                nc.vector.tensor_reduce(den[:, HG:], ev[:, HG:], AX.X, ALU.add)
                num = numall[:, ci * GRP:(ci + 1) * GRP]
                w3 = w[:].rearrange("p (g b) -> p g b", b=BASE)
                nc.vector.tensor_reduce(num[:, :HG], w3[:, :HG], AX.X, ALU.add)
                nc.vector.tensor_reduce(num[:, HG:], w3[:, HG:], AX.X, ALU.add)
                rcp = sp.tile([P, GRP], F32, tag="rcp")
                scr = sp.tile([P, GRP], F32, tag="scr")
                nc.vector.reciprocal_approx_accurate(rcp[:], den[:], scr[:])
                return rcp

            def p1_tail(ci, rcp):
                """expected value + pow-weighted sum (Pool) + clip (DVE)."""
                num = numall[:, ci * GRP:(ci + 1) * GRP]
                ex = sp.tile([P, GRP], F32, tag="ex")
                nc.gpsimd.tensor_tensor(ex[:], num, rcp[:], ALU.mult)
                ex3 = ex[:].rearrange("p (r d) -> p r d", d=DS)
                v4 = sp.tile([P, CH * NI * 4], F32, tag="v4")
                v43 = v4[:].rearrange("p (r d) -> p r d", d=4)
                nc.gpsimd.tensor_tensor(v43, ex3[:, :, 0:4], ex3[:, :, 4:8], ALU.add)
                v2 = sp.tile([P, CH * NI * 2], F32, tag="v2")
                v23 = v2[:].rearrange("p (r d) -> p r d", d=2)
                nc.gpsimd.tensor_tensor(v23, v43[:, :, 0:2], v43[:, :, 2:4], ALU.add)
                vm = sp.tile([P, CH * NI], F32, tag="vm")
                nc.gpsimd.tensor_tensor(vm[:], v23[:, :, 0], v23[:, :, 1], ALU.add)
                nc.vector.tensor_scalar(
                    vmag[:, ci * CH * NI:(ci + 1) * CH * NI], vm[:],
                    MAG_MIN, MAG_MAX, ALU.max, ALU.min,
                )

            p1_fd(0)
            p1_fp(0)
            p1_e(0)
            p1_fd(1)
            # phase-2 inputs after the first chunk DMAs so they don't block them
            nc.sync.dma_start(otile[:], od)
            nc.sync.dma_start(wsign[:], vsg)
            nc.sync.dma_start(gtile[:], gd)
            nc.scalar.activation(absO2[:], otile[:], ACTF.Abs, scale=2.0)
            nc.vector.tensor_scalar(onemg[:], gtile[:], -1.0, 1.0, ALU.mult, ALU.add)
            for ci in range(NCHUNK):
                p1_bp(ci)
                rcp = p1_bd(ci)
                if ci + 1 < NCHUNK:
                    p1_fp(ci + 1)
                    p1_e(ci + 1)
                if ci + 2 < NCHUNK:
                    p1_fd(ci + 2)
                p1_tail(ci, rcp)

            # ---- phase 2: DAG recurrence ----
            wm3 = wmag[:].rearrange("p (t n) -> p t n", n=TN)
            nc.vector.tensor_copy(
                wm3[:, :, 0:NI], vmag[:].rearrange("p (t n) -> p t n", n=NI)
            )
            nc.vector.tensor_tensor(signed[:], wsign[:], wmag[:], ALU.mult)
            _emit_ln(nc, stp, logm[:], wmag[:], TP * TN, "lni")
            dc3 = diffc[:].rearrange("p (t n) -> p t n", n=TN)
            nc.vector.tensor_tensor(diffc[:], signed[:], logm[:], ALU.subtract)
            ws3 = wsign[:].rearrange("p (t n) -> p t n", n=TN)
            sg3 = signed[:].rearrange("p (t n) -> p t n", n=TN)
            lg3 = logm[:].rearrange("p (t n) -> p t n", n=TN)
            o4 = otile[:].rearrange("p (t s n) -> p t s n", s=DD, n=TN)
            a4 = absO2[:].rearrange("p (t s n) -> p t s n", s=DD, n=TN)


            def gs(ap, s, n_bcast=None):
                v = ap[:, s::DD]
                if n_bcast is None:
                    return v
                return v.unsqueeze(2).broadcast_to((P, TP, n_bcast))

            def p2_front(s):
                """Heavy dots over nodes [0, K): everything but the node written
                by step s-1.  Depends only on >=2-step-old state, so it runs one
                step ahead of its consumer."""
                K = NI if s == 0 else NI - 1 + s
                t1 = stp.tile([P, TP * TN], F32, tag="t1f")
                t13 = t1[:].rearrange("p (t n) -> p t n", n=TN)
                nc.gpsimd.tensor_tensor(
                    t13[:, :, :K], dc3[:, :, :K], gs(gtile[:], s, K), ALU.mult
                )
                mx = stp.tile([P, TP * TN], F32, tag="mxf")
                mx3 = mx[:].rearrange("p (t n) -> p t n", n=TN)
                nc.vector.tensor_tensor(
                    mx3[:, :, :K], lg3[:, :, :K], t13[:, :, :K], ALU.add
                )
                rt = stp.tile([P, TP * TN], F32, tag="rtf")
                rt3 = rt[:].rearrange("p (t n) -> p t n", n=TN)
                nc.vector.tensor_tensor(
                    rt3[:, :, :K], mx3[:, :, :K], o4[:, :, s, :K], ALU.mult
                )
                rold = st2.tile([P, TP], F32, tag="rold")
                nc.vector.tensor_reduce(rold[:], rt3[:, :, :K], AX.X, ALU.add)

                sw = stp.tile([P, TP * TN], F32, tag="swf")
                sw3 = sw[:].rearrange("p (t n) -> p t n", n=TN)
                nc.gpsimd.tensor_tensor(
                    sw3[:, :, :K], ws3[:, :, :K], a4[:, :, s, :K], ALU.mult
                )
                swp = stp.tile([P, TP * TN], F32, tag="swpf")
                swp3 = swp[:].rearrange("p (t n) -> p t n", n=TN)
                nc.scalar.activation(swp3[:, :, :K], sw3[:, :, :K], ACTF.Copy, bias=1.0)
                # product over the K nodes: pairwise multiply tree on Pool
                ta = st2.tile([P, TP * TN], F32, tag="ta")
                tb = st2.tile([P, TP * TN], F32, tag="tb")
                pbufs = [
                    ta[:].rearrange("p (t n) -> p t n", n=TN),
                    tb[:].rearrange("p (t n) -> p t n", n=TN),
                ]
                src3, width, pi = swp3, K, 0
                while width > 1:
                    half, odd = width // 2, width % 2
                    dst3 = pbufs[pi]
                    nc.gpsimd.tensor_tensor(
                        dst3[:, :, :half], src3[:, :, :half],
                        src3[:, :, half:2 * half], ALU.mult,
                    )
                    if odd:
                        nc.vector.tensor_copy(dst3[:, :, half], src3[:, :, 2 * half])
                    src3, width, pi = dst3, half + odd, 1 - pi
                return rold, src3[:, :, 0]

            res = sp.tile([P, TP], F32, tag="res")
            fr = p2_front(0)

            for s in range(DD):
                last = s == DD - 1
                rold, prodold = fr

                rp = st2.tile([P, 2 * TP], F32, tag="rp")  # [R | prod]
                if s == 0:
                    nc.vector.tensor_copy(rp[:, 0:TP], rold[:])
                    nc.vector.tensor_copy(rp[:, TP:2 * TP], prodold)
                else:
                    # fold in the node written by step s-1 (index NI-1+s)
                    nd = NI - 1 + s
                    q1 = stp.tile([P, TP], F32, tag="q1")
                    nc.vector.tensor_tensor(
                        q1[:], lg3[:, :, nd], gs(onemg[:], s), ALU.mult
                    )
                    q2 = stp.tile([P, TP], F32, tag="q2")
                    nc.vector.tensor_tensor(
                        q2[:], sg3[:, :, nd], gs(gtile[:], s), ALU.mult
                    )
                    mixn = stp.tile([P, TP], F32, tag="mixn")
                    nc.vector.tensor_tensor(mixn[:], q1[:], q2[:], ALU.add)
                    rn = stp.tile([P, TP], F32, tag="rn")
                    nc.vector.tensor_tensor(rn[:], mixn[:], o4[:, :, s, nd], ALU.mult)
                    nc.vector.tensor_tensor(rp[:, 0:TP], rold[:], rn[:], ALU.add)
                    swn = stp.tile([P, TP], F32, tag="swn")
                    nc.vector.tensor_tensor(swn[:], ws3[:, :, nd], a4[:, :, s, nd], ALU.mult)
                    nc.vector.tensor_scalar(swn[:], swn[:], 1.0, None, ALU.add)
                    nc.vector.tensor_tensor(rp[:, TP:2 * TP], prodold, swn[:], ALU.mult)

                # prefetch next step's heavy dots while this step's tail runs
                if not last:
                    fr = p2_front(s + 1)

                # tanh(y/SIGN_TEMP) = 1 - 2/(1+exp(2e4*y)) on [R | prod] at once
                yc = stp.tile([P, 2 * TP], F32, tag="yc")
                nc.vector.tensor_scalar(yc[:], rp[:], -0.005, 0.005, ALU.max, ALU.min)
                tE = stp.tile([P, 2 * TP], F32, tag="tE")
                nc.scalar.activation(tE[:], yc[:], ACTF.Exp, scale=2.0 * INV_SIGN_TEMP)
                u = stp.tile([P, 2 * TP], F32, tag="u")
                nc.vector.tensor_scalar(u[:], tE[:], 1.0, 1e30, ALU.add, ALU.min)
                rc2 = stp.tile([P, 2 * TP], F32, tag="rc2")
                sc2 = stp.tile([P, 2 * TP], F32, tag="sc2")
                nc.vector.reciprocal_approx_accurate(rc2[:], u[:], sc2[:])
                th = stp.tile([P, 2 * TP], F32, tag="th")
                nc.vector.tensor_scalar(th[:], rc2[:], -2.0, 1.0, ALU.mult, ALU.add)
                lin_sign, log_sign = th[:, 0:TP], th[:, TP:2 * TP]

                # s_new = G*lin + (1-G)*log  (clip dropped: convex combo of
                # values in [-1,1] stays within 1ulp of the range)
                sa = stp.tile([P, TP], F32, tag="sa")
                nc.vector.tensor_tensor(sa[:], lin_sign, gs(gtile[:], s), ALU.mult)
                sb = stp.tile([P, TP], F32, tag="sb")
                nc.vector.tensor_tensor(sb[:], log_sign, gs(onemg[:], s), ALU.mult)
                snew = stp.tile([P, TP], F32, tag="snew")
                nc.vector.tensor_tensor(snew[:], sa[:], sb[:], ALU.add)
                nc.vector.tensor_scalar(snew[:], snew[:], -1.0, 1.0, ALU.max, ALU.min)

                # m_new = G*min(|R|,MAX) + (1-G)*exp(clip(R,-100,EXP_CLAMP)), clipped.
                # The EXP_CLAMP upper bound replaces the reference's inf path: any
                # clamped value yields (1-G)*e^88.72 >= 2e31, which the final clip
                # maps to 1e28 exactly as inf would.
                R = rp[:, 0:TP]
                absR = stp.tile([P, TP], F32, tag="absR")
                nc.scalar.activation(absR[:], R, ACTF.Abs)
                nc.vector.tensor_scalar(absR[:], absR[:], MAG_MAX, None, ALU.min)
                rc = stp.tile([P, TP], F32, tag="rc")
                nc.vector.tensor_scalar(rc[:], R, -LOG_LIM, EXP_CLAMP, ALU.max, ALU.min)
                logres = stp.tile([P, TP], F32, tag="logres")
                nc.scalar.activation(logres[:], rc[:], ACTF.Exp)
                ma = stp.tile([P, TP], F32, tag="ma")
                nc.vector.tensor_tensor(ma[:], absR[:], gs(gtile[:], s), ALU.mult)
                mb2 = stp.tile([P, TP], F32, tag="mb2")
                nc.vector.tensor_tensor(mb2[:], logres[:], gs(onemg[:], s), ALU.mult)
                mnew = stp.tile([P, TP], F32, tag="mnew")
                nc.vector.tensor_tensor(mnew[:], ma[:], mb2[:], ALU.add)
                nc.vector.tensor_scalar(mnew[:], mnew[:], MAG_MIN, MAG_MAX, ALU.max, ALU.min)

                if last:
                    # output = wsign[16] * wmag[16]; skip the state writes
                    nc.vector.tensor_tensor(res[:], mnew[:], snew[:], ALU.mult)
                else:
                    ni = NI + s
                    nc.vector.tensor_copy(wm3[:, :, ni], mnew[:])
                    nc.vector.tensor_copy(ws3[:, :, ni], snew[:])
                    nc.gpsimd.tensor_tensor(sg3[:, :, ni], mnew[:], snew[:], ALU.mult)
                    _emit_ln(nc, stp, lg3[:, :, ni], mnew[:], TP, "lns")
                    nc.vector.tensor_tensor(
                        dc3[:, :, ni], sg3[:, :, ni], lg3[:, :, ni], ALU.subtract
                    )

            nc.sync.dma_start(out, res[:])

    nc.compile()
    return nc


_NC_CACHE = None


def _get_nc():
    global _NC_CACHE
    if _NC_CACHE is None:
        _NC_CACHE = build_program()
    return _NC_CACHE


def make_in_maps(digit_logits, V_sign, O, G):
    dlf = np.ascontiguousarray(digit_logits, dtype=np.float32).reshape(B * T, PTOK)
    vsf = np.ascontiguousarray(V_sign, dtype=np.float32).reshape(B * T, TN)
    of = np.ascontiguousarray(O, dtype=np.float32).reshape(B * T, DD * TN)
    gf = np.ascontiguousarray(G, dtype=np.float32).reshape(B * T, DD)
    pat = np.zeros(DS * BASE, dtype=np.float32)
    for dd in range(DS):
        for i in range(BASE):
            pat[dd * BASE + i] = i * POWERS[dd]
    wpat = np.tile(pat[None, :], (P, 1))
    in_maps = []
    for c in range(NCORES):
        s0, s1 = c * TOK_PER_CORE, (c + 1) * TOK_PER_CORE
        in_maps.append({
            "dl": np.ascontiguousarray(dlf[s0:s1].reshape(P, TP * PTOK)),
            "vsg": np.ascontiguousarray(vsf[s0:s1].reshape(P, TP * TN)),
            "od": np.ascontiguousarray(of[s0:s1].reshape(P, TP * DD * TN)),
            "gd": np.ascontiguousarray(gf[s0:s1].reshape(P, TP * DD)),
            "wpat": wpat,
        })
    return in_maps


def kernel(digit_logits, V_sign, O, G, _trace=False, _return_results=False):
    nc = _get_nc()
    in_maps = make_in_maps(digit_logits, V_sign, O, G)
    res = run_bass_kernel_spmd(nc, in_maps, list(range(NCORES)), trace=_trace)
    outs = [np.asarray(res.results[c]["out"]).reshape(TOK_PER_CORE) for c in range(NCORES)]
    full = np.concatenate(outs).reshape(B, T)
    if _return_results:
        return full, res
    return full
